# revision 4
# baseline (speedup 1.0000x reference)
"""BiMPM kernel for Trainium2 — v3.

Structure vs v2 baseline:
- Attentive-max loop feeds broadcasts from HOST-REPLICATED rows in DRAM
  (c1rep/c2rep: every SBUF partition has its own copy of every row), so
  one grouped dma_start loads 8 j-rows broadcast to all 128 partitions at
  near line rate. PE one-hot broadcast matmuls and their PSUM drains are
  gone.
- Groups of 8 j's: 'A' groups multiply on ACT (scalar.mul, per-partition
  cos column as scale), 'D' groups on DVE (tensor_scalar 4x). One 8-wide
  DVE tensor_tensor max per group into ping-pong accumulators.
- Side 1 (amx2) completes before side 2 finishes, so its mpm feature
  block overlaps side 2's loop; mm/mpm/att_mean blocks are interleaved
  between groups to fill engine gaps.

Self-contained: hardcodes B=8, L=128, H=768, P=16.
"""
import sys

sys.path.insert(0, "/opt/trn_rl_repo")

import numpy as np
import ml_dtypes
from contextlib import ExitStack

from concourse import bacc, mybir, masks
import concourse.tile as tile
from concourse.bass_utils import run_bass_kernel_spmd
from concourse.bass import MemorySpace

B, L, H, PP, NCH, NF = 8, 128, 768, 16, 6, 102
EPS = 1e-8
F32 = mybir.dt.float32
BF16 = mybir.dt.bfloat16
FP8 = mybir.dt.float8e4
AX = mybir.AxisListType
OP = mybir.AluOpType
AF = mybir.ActivationFunctionType

BLK_ATT = slice(0, 17)    # w2ab columns: [ones|att16 | ones|matt16]
BLK_MATT = slice(17, 34)

# per-8-group mul engine: A = ACT (scalar.mul), D = DVE (tensor_scalar)
PATTERN8 = "AADAADAD"
G = 8  # group width (j's per broadcast load / max op)


def _groups(jmax):
    return [list(range(g, min(g + G, jmax))) for g in range(0, jmax, G)]


class _AttMax:
    """Emits one side's attentive-max loop in resumable chunks."""

    def __init__(self, nc, pools, rep_dram, cosMcols, jmax, tag, qsel):
        self.nc = nc
        self.sb, self.tbp, self.bcp = pools
        self.rep = rep_dram
        self.cols = cosMcols
        self.groups = _groups(jmax)
        self.tag = tag
        self.qsel = qsel
        self.accs = []
        for k in range(2):
            acc = self.sb.tile([L, G, H], BF16, tag=f"{tag}_acc{k}")
            self.accs.append(acc)
        self.step = 0

    def emit(self, n):
        nc = self.nc
        while n > 0 and self.step < len(self.groups):
            g = self.step
            js = self.groups[g]
            j0, cnt = js[0], len(js)
            kind = PATTERN8[(g + self.qsel) % len(PATTERN8)]
            bc8 = self.bcp.tile([L, G, H], BF16, tag="bc8")
            eng = nc.sync if (g + self.qsel) % 2 == 0 else nc.scalar
            eng.dma_start(
                bc8[:, 0:cnt, :], self.rep[:, j0 * H:(j0 + cnt) * H])
            first = g == 0
            tb8 = self.accs[0] if first else self.tbp.tile(
                [L, G, H], BF16, tag="tb8")
            for t, j in enumerate(js):
                if kind == "A":
                    nc.scalar.mul(tb8[:, t, :], bc8[:, t, :],
                                  self.cols[:, j:j + 1])
                else:
                    nc.vector.tensor_scalar(
                        tb8[:, t, :], bc8[:, t, :], self.cols[:, j:j + 1],
                        None, OP.mult)
            if cnt < G:
                nc.vector.memset(tb8[:, cnt:G, :], -1e30)
            if not first:
                prev = self.accs[(self.step + 1) % 2]
                nxt = self.accs[self.step % 2]
                nc.vector.tensor_tensor(nxt[:], prev[:], tb8[:], op=OP.max)
            self.step += 1
            n -= 1
        return self.step >= len(self.groups)

    def fold(self):
        nc = self.nc
        fin = self.accs[1 - self.step % 2] if self.step > 1 else self.accs[0]
        h4 = self.sb.tile([L, 4, H], BF16, tag=self.tag + "_h4")
        nc.vector.tensor_tensor(h4[:], fin[:, 0:4, :], fin[:, 4:8, :],
                                op=OP.max)
        h2 = self.sb.tile([L, 2, H], BF16, tag=self.tag + "_h2")
        nc.vector.tensor_tensor(h2[:], h4[:, 0:2, :], h4[:, 2:4, :],
                                op=OP.max)
        out = self.sb.tile([L, H], BF16, tag=self.tag + "_o")
        nc.vector.tensor_tensor(out[:], h2[:, 0, :], h2[:, 1, :], op=OP.max)
        return out


def _trace_kernel(tc, dins, dout, jmax1, jmax2):
    nc = tc.nc
    with ExitStack() as ctx:
        sb = ctx.enter_context(tc.tile_pool(name="sb", bufs=1))
        sc = ctx.enter_context(tc.tile_pool(name="sc", bufs=3))
        tbp = ctx.enter_context(tc.tile_pool(name="tbp", bufs=2))
        bcp = ctx.enter_context(tc.tile_pool(name="bcp", bufs=3))
        ps_b = ctx.enter_context(
            tc.tile_pool(name="ps_b", bufs=2, space=MemorySpace.PSUM))
        ps_mm = ctx.enter_context(
            tc.tile_pool(name="ps_mm", bufs=1, space=MemorySpace.PSUM))
        ps_t = ctx.enter_context(
            tc.tile_pool(name="ps_t", bufs=1, space=MemorySpace.PSUM))
        ps_w = ctx.enter_context(
            tc.tile_pool(name="ps_w", bufs=1, space=MemorySpace.PSUM))

        def load(name, shape, dt=F32, rearr=None, eng=None, **kw):
            t = sb.tile(shape, dt, tag=name)
            src = dins[name][:]
            if rearr is not None:
                src = src.rearrange(rearr, **kw)
            (eng or nc.sync).dma_start(t[:], src)
            return t

        c1T = load("c1T", [L, NCH, L], BF16, "(c p) n -> p c n", p=L)
        c2T = load("c2T", [L, NCH, L], BF16, "(c p) n -> p c n", p=L)
        rw1 = load("rw1", [L, 68])
        rw2 = load("rw2", [L, 68])
        invl1 = load("invl1", [L, 1])
        invl2 = load("invl2", [L, 1])
        mone1b = load("mone1b", [L, L], eng=nc.scalar)
        mone2b = load("mone2b", [L, L], eng=nc.scalar)
        c1b = load("c1b", [L, H], BF16, eng=nc.scalar)
        c2b = load("c2b", [L, H], BF16, eng=nc.scalar)
        rhs1 = load("rhs1", [L, NCH, 34], BF16, "(c p) n -> p c n", p=L)
        rhs2 = load("rhs2", [L, NCH, 34], BF16, "(c p) n -> p c n", p=L)
        w2ab = load("w2ab", [L, NCH, 34], BF16, "(c p) n -> p c n", p=L)
        rw1mp = load("rw1mp", [L, PP])
        rw2mp = load("rw2mp", [L, PP])

        ident = sb.tile([L, L], F32, tag="ident")
        masks.make_identity(nc, ident[:])
        identb = sb.tile([L, L], BF16, tag="identb")
        masks.make_identity(nc, identb[:])
        ones_col = sb.tile([L, 1], F32, tag="ones_col")
        nc.vector.memset(ones_col[:], 1.0)

        out1 = sb.tile([L, NF], F32, tag="out1")
        out2 = sb.tile([L, NF], F32, tag="out2")

        # ---- cos chain ----
        dotsp = ps_t.tile([L, L], F32, tag="t")
        for c in range(NCH):
            nc.tensor.matmul(dotsp[:], c1T[:, c, :], c2T[:, c, :],
                             start=(c == 0), stop=(c == NCH - 1))
        wS = sc.tile([L, L], F32, tag="wS")
        nc.vector.tensor_scalar(wS[:], dotsp[:], rw1[:, 0:1], None, OP.mult)
        wTp = ps_t.tile([L, L], F32, tag="t")
        nc.tensor.transpose(wTp[:], wS[:], ident[:])
        cosT = sb.tile([L, L], F32, tag="cosT")
        nc.vector.tensor_scalar(cosT[:], wTp[:], rw2[:, 0:1], None, OP.mult)
        cosp = ps_t.tile([L, L], F32, tag="t")
        nc.tensor.transpose(cosp[:], cosT[:], ident[:])
        cos = sb.tile([L, L], F32, tag="cos")
        nc.scalar.copy(cos[:], cosp[:])
        # bf16 copies for att_mean matmul lhsT
        cosb = sb.tile([L, L], BF16, tag="cosb")
        nc.vector.tensor_copy(cosb[:], cosp[:])
        cosTb = sb.tile([L, L], BF16, tag="cosTb")
        nc.vector.tensor_copy(cosTb[:], cosT[:])

        # ---- cosM / cosMT (+1.0 in invalid columns) ----
        cosM = sb.tile([L, L], F32, tag="cosM")
        nc.vector.tensor_tensor(cosM[:], cosp[:], mone2b[:], op=OP.add)
        cosMT = sb.tile([L, L], F32, tag="cosMT")
        nc.vector.tensor_tensor(cosMT[:], cosT[:], mone1b[:], op=OP.add)

        # ---- attentive mean (softmax 1/sum cancels in cosine features) ----
        def att_mean(lhsT, rhs, tag):
            sp = ps_b.tile([L, H], F32, tag="bc")
            nc.tensor.matmul(sp[:, 0:512], lhsT[:], rhs[:, 0:512],
                             start=True, stop=True)
            nc.tensor.matmul(sp[:, 512:H], lhsT[:], rhs[:, 512:H],
                             start=True, stop=True)
            am = sb.tile([L, H], BF16, tag=tag)
            nc.scalar.activation(am[:], sp[:], AF.Exp, scale=1.0)
            return am

        am2 = att_mean(cosTb, c2b, "am2")   # [i,H]
        am1 = att_mean(cosb, c1b, "am1")    # [j,H]

        # ---- cmax / cmean ----
        def cmaxmean(cosA, cosB, invl, out):
            nc.vector.reduce_max(out[:, 0:1], cosA[:], axis=AX.X)
            mp = ps_w.tile([L, 34], F32, tag="wa")
            nc.tensor.matmul(mp[:, 0:1], cosB[:], ones_col[:], start=True,
                             stop=True)
            nc.vector.tensor_scalar(out[:, 1:2], mp[:, 0:1], invl[:, 0:1],
                                    None, OP.mult)

        cmaxmean(cos, cosT, invl2, out1)
        cmaxmean(cosT, cos, invl1, out2)

        # ---- ff/bf matvec features ----
        def ff_feats(cT, rhs, rw, out):
            ffp = ps_w.tile([L, 34], F32, tag="wa")
            for c in range(NCH):
                nc.tensor.matmul(ffp[:], cT[:, c, :], rhs[:, c, :],
                                 start=(c == 0), stop=(c == NCH - 1))
            nc.vector.tensor_tensor(out[:, 2:36], ffp[:], rw[:, 0:34],
                                    op=OP.mult)

        ff_feats(c1T, rhs1, rw1, out1)
        ff_feats(c2T, rhs2, rw2, out2)

        # ---- attentive max loops (chunked emission) ----
        att1 = _AttMax(nc, (sb, tbp, bcp), dins["c2rep"][:], cosM,
                       jmax2, "amx2", 0)
        att2 = _AttMax(nc, (sb, tbp, bcp), dins["c1rep"][:], cosMT,
                       jmax1, "amx1", 1)

        # ---- am/amx rowwise mpm feature blocks ----
        def mpm_block(v, cT, rw_side, blk, out, col0):
            vsqT = sc.tile([L, NCH, L], BF16, tag="vsqT")
            prT = sc.tile([L, NCH, L], BF16, tag="prT")
            for c in range(NCH):
                tp = ps_t.tile([L, L], BF16, tag="t")
                nc.tensor.transpose(tp[:], v[:, c * L:(c + 1) * L], identb[:])
                nc.scalar.square(vsqT[:, c, :], tp[:])
                nc.vector.tensor_tensor(prT[:, c, :], cT[:, c, :], tp[:],
                                        op=OP.mult)
            numpt = ps_w.tile([L, 34], F32, tag="wa")
            wnpt = ps_w.tile([L, 34], F32, tag="wb")
            nump = numpt[:, 0:17]
            wnp = wnpt[:, 0:17]
            for c in range(NCH):
                nc.tensor.matmul(nump[:], prT[:, c, :], w2ab[:, c, blk],
                                 start=(c == 0), stop=(c == NCH - 1))
            for c in range(NCH):
                nc.tensor.matmul(wnp[:], vsqT[:, c, :], w2ab[:, c, blk],
                                 start=(c == 0), stop=(c == NCH - 1))
            rwv = sc.tile([L, 17], F32, tag="rwv")
            nc.scalar.sqrt(rwv[:], wnp[:])
            nc.vector.tensor_scalar(rwv[:], rwv[:], EPS, None, OP.max)
            nc.vector.reciprocal(rwv[:], rwv[:])
            ft = sc.tile([L, 17], F32, tag="ft")
            nc.vector.tensor_tensor(ft[:], nump[:],
                                    rw_side[:, 34 + blk.start:34 + blk.stop],
                                    op=OP.mult)
            nc.vector.tensor_tensor(out[:, col0:col0 + 17], ft[:], rwv[:],
                                    op=OP.mult)

        # ---- mm (pairwise multi-perspective) twin-matmul block ----
        mmr1 = load("mmr1", [L, NCH, PP * L], BF16, "(c p) n -> p c n", p=L,
                    eng=nc.scalar)
        mmr2 = load("mmr2", [L, NCH, PP * L], BF16, "(c p) n -> p c n", p=L)
        mme1 = load("mme1", [L, NCH, PP], BF16, "(c p) n -> p c n", p=L)
        mme2 = load("mme2", [L, NCH, PP], BF16, "(c p) n -> p c n", p=L)

        def mm_qtr(cT, mmr, rwmp, out, qtr):
            o = ps_mm.tile([L, 4, L], F32, tag="mm")
            qs = slice(qtr * 4 * L, (qtr + 1) * 4 * L)
            for c in range(NCH):
                nc.tensor.matmul(o[:], cT[:, c, :], mmr[:, c, qs],
                                 start=(c == 0), stop=(c == NCH - 1))
            mx = sc.tile([L, 4], F32, tag="mx4")
            nc.vector.reduce_max(mx[:, :, None], o[:], axis=AX.X)
            nc.vector.tensor_tensor(
                out[:, 36 + qtr * 4:36 + (qtr + 1) * 4], mx[:],
                rwmp[:, qtr * 4:(qtr + 1) * 4], op=OP.mult)

        def mm_mean(cT, mme, rwmp, out):
            mnt = ps_w.tile([L, 34], F32, tag="wa")
            mn = mnt[:, 0:PP]
            for c in range(NCH):
                nc.tensor.matmul(mn[:], cT[:, c, :], mme[:, c, :],
                                 start=(c == 0), stop=(c == NCH - 1))
            nc.vector.tensor_tensor(out[:, 52:68], mn[:], rwmp[:],
                                    op=OP.mult)

        # ---- interleaved emission ----
        # side 1 groups with am-feature blocks and mm qtrs as gap fillers
        att1.emit(3)
        mpm_block(am2, c1T, rw1, BLK_ATT, out1, 68)
        att1.emit(3)
        mpm_block(am1, c2T, rw2, BLK_ATT, out2, 68)
        att1.emit(3)
        mm_qtr(c1T, mmr1, rw1mp, out1, 0)
        att1.emit(2)
        mm_qtr(c1T, mmr1, rw1mp, out1, 1)
        att1.emit(2)
        mm_qtr(c1T, mmr1, rw1mp, out1, 2)
        att1.emit(2)
        mm_qtr(c1T, mmr1, rw1mp, out1, 3)
        att1.emit(99)
        mm_mean(c1T, mme1, rw1mp, out1)

        att2.emit(4)
        amx2 = att1.fold()
        att2.emit(2)
        mm_qtr(c2T, mmr2, rw2mp, out2, 0)
        att2.emit(2)
        mpm_block(amx2, c1T, rw1, BLK_MATT, out1, 85)
        att2.emit(2)
        mm_qtr(c2T, mmr2, rw2mp, out2, 1)
        att2.emit(2)
        mm_qtr(c2T, mmr2, rw2mp, out2, 2)
        att2.emit(2)
        mm_qtr(c2T, mmr2, rw2mp, out2, 3)
        att2.emit(99)
        mm_mean(c2T, mme2, rw2mp, out2)
        amx1 = att2.fold()
        mpm_block(amx1, c2T, rw2, BLK_MATT, out2, 85)

        # ---- store ----
        nc.sync.dma_start(dout[0:L, 0:85], out1[:, 0:85])
        nc.sync.dma_start(dout[L:2 * L, 0:85], out2[:, 0:85])
        nc.sync.dma_start(dout[0:L, 85:NF], out1[:, 85:NF])
        nc.sync.dma_start(dout[L:2 * L, 85:NF], out2[:, 85:NF])


_CACHED = {}


def _build(jmax1, jmax2):
    key = (jmax1, jmax2)
    if key in _CACHED:
        return _CACHED[key]
    nc = bacc.Bacc("TRN2", target_bir_lowering=False, debug=False,
                   enable_asserts=False)
    dins = {}
    for name, shape, dt in [
            ("c1b", [L, H], BF16), ("c2b", [L, H], BF16),
            ("c1T", [H, L], BF16), ("c2T", [H, L], BF16),
            ("c1rep", [L, L * H], BF16), ("c2rep", [L, L * H], BF16),
            ("rhs1", [H, 34], BF16), ("rhs2", [H, 34], BF16),
            ("w2ab", [H, 34], BF16),
            ("mmr1", [H, PP * L], BF16), ("mmr2", [H, PP * L], BF16),
            ("mme1", [H, PP], BF16), ("mme2", [H, PP], BF16),
            ("rw1", [L, 68], F32), ("rw2", [L, 68], F32),
            ("rw1mp", [L, PP], F32), ("rw2mp", [L, PP], F32),
            ("mone1b", [L, L], F32), ("mone2b", [L, L], F32),
            ("invl1", [L, 1], F32), ("invl2", [L, 1], F32)]:
        dins[name] = nc.dram_tensor(name, shape, dt, kind="ExternalInput")
    dout = nc.dram_tensor("out", [2 * L, NF], F32, kind="ExternalOutput")
    with tile.TileContext(nc) as tc:
        _trace_kernel(tc, dins, dout[:], jmax1, jmax2)
    nc.compile()
    _CACHED[key] = nc
    return nc


def _host_prep(c1raw, m1, c2raw, m2, w_ff, w_fb, w_mp, w_att, w_matt):
    asb = lambda a: np.ascontiguousarray(a, dtype=ml_dtypes.bfloat16)
    asf = lambda a: np.ascontiguousarray(a, dtype=np.float32)

    c1 = (c1raw * m1[:, None]).astype(np.float32)
    c2 = (c2raw * m2[:, None]).astype(np.float32)
    len1, len2 = float(m1.sum()), float(m2.sum())
    lp1, lp2 = max(int(len1) - 1, 0), max(int(len2) - 1, 0)

    def mpm_rhs(v, w):
        w2 = w * w
        rn = 1.0 / max(np.sqrt((v * v).sum()), EPS)
        wn = np.sqrt((w2 * (v * v)[None, :]).sum(1))
        rwn = 1.0 / np.maximum(wn, EPS)
        return np.concatenate(
            [(v * rn)[:, None], (w2 * v[None, :] * rwn[:, None]).T], 1)

    rhs1 = np.concatenate([mpm_rhs(c2[lp2], w_ff), mpm_rhs(c2[0], w_fb)], 1)
    rhs2 = np.concatenate([mpm_rhs(c1[lp1], w_ff), mpm_rhs(c1[0], w_fb)], 1)

    # rw tables [L, 68]: [ones|ff16 | ones|fb16 | ones|att16 | ones|matt16]
    ones_h = np.ones((1, H), np.float32)
    wsq = np.concatenate([ones_h, w_ff**2, ones_h, w_fb**2,
                          ones_h, w_att**2, ones_h, w_matt**2], 0)  # [68,H]
    def rw_of(c):
        wn = np.sqrt(wsq @ (c * c).T)           # [68, L]
        return (1.0 / np.maximum(wn, EPS)).T    # [L, 68]
    rw1, rw2 = rw_of(c1), rw_of(c2)

    # mp folding
    w2mp = (w_mp * w_mp).astype(np.float32)          # [P, H]
    def rwmp_of(c):
        wn = np.sqrt(w2mp @ (c * c).T)               # [P, L]
        return 1.0 / np.maximum(wn, EPS)             # [P, L]
    rw1mp_t, rw2mp_t = rwmp_of(c1), rwmp_of(c2)      # [P, L]
    # mmr1[h, (p, j)] = w2mp[p,h] * c2[j,h] * rw2mp[p,j]
    mmr1 = np.einsum("ph,jh,pj->hpj", w2mp, c2, rw2mp_t).reshape(H, PP * L)
    mmr2 = np.einsum("ph,ih,pi->hpi", w2mp, c1, rw1mp_t).reshape(H, PP * L)
    # mme1[h, p] = w2mp[p,h] * (sum_j c2[j,h] rw2mp[p,j]) / len2
    s2 = np.einsum("jh,pj->ph", c2, rw2mp_t)
    s1 = np.einsum("ih,pi->ph", c1, rw1mp_t)
    mme1 = (w2mp * s2 / max(len2, EPS)).T            # [H, P]
    mme2 = (w2mp * s1 / max(len1, EPS)).T

    w2ab = np.concatenate([ones_h, w_att**2, ones_h, w_matt**2], 0).T  # [H,34]

    # broadcast-replicated masked rows: every partition holds every row
    c1x = asb(c1 + ((m1 - 1) * 1e30)[:, None]).reshape(-1)
    c2x = asb(c2 + ((m2 - 1) * 1e30)[:, None]).reshape(-1)
    c1rep = np.ascontiguousarray(np.broadcast_to(c1x[None, :], (L, L * H)))
    c2rep = np.ascontiguousarray(np.broadcast_to(c2x[None, :], (L, L * H)))

    bc = lambda r: np.ascontiguousarray(
        np.broadcast_to(r[None, :], (L, L)), dtype=np.float32)
    return dict(
        c1b=asb(c1), c2b=asb(c2),
        c1rep=c1rep, c2rep=c2rep,
        c1T=asb(c1.T), c2T=asb(c2.T),
        rhs1=asb(rhs1), rhs2=asb(rhs2), w2ab=asb(w2ab),
        mmr1=asb(mmr1), mmr2=asb(mmr2), mme1=asb(mme1), mme2=asb(mme2),
        rw1=asf(rw1), rw2=asf(rw2),
        rw1mp=asf(rw1mp_t.T), rw2mp=asf(rw2mp_t.T),
        mone1b=bc(1 - m1), mone2b=bc(1 - m2),
        invl1=np.full((L, 1), 1.0 / max(len1, EPS), np.float32),
        invl2=np.full((L, 1), 1.0 / max(len2, EPS), np.float32),
    )


def kernel(context_1, mask_1, context_2, mask_2,
           w_ff, w_fb, w_mp, w_att, w_matt, **_unused):
    context_1 = np.asarray(context_1, dtype=np.float32)
    context_2 = np.asarray(context_2, dtype=np.float32)
    mask_1 = np.asarray(mask_1, dtype=np.float32)
    mask_2 = np.asarray(mask_2, dtype=np.float32)
    w_ff, w_fb = np.asarray(w_ff, np.float32), np.asarray(w_fb, np.float32)
    w_mp = np.asarray(w_mp, np.float32)
    w_att, w_matt = np.asarray(w_att, np.float32), np.asarray(w_matt, np.float32)
    assert context_1.shape == (B, L, H), context_1.shape

    jmax1 = int(mask_1.sum(1).max())
    jmax2 = int(mask_2.sum(1).max())
    nc = _build(jmax1, jmax2)
    in_maps = [
        _host_prep(context_1[b], mask_1[b], context_2[b], mask_2[b],
                   w_ff, w_fb, w_mp, w_att, w_matt)
        for b in range(B)
    ]
    res = run_bass_kernel_spmd(nc, in_maps, core_ids=list(range(B)))
    global LAST_RESULTS
    LAST_RESULTS = res
    return np.stack([res.results[b]["out"] for b in range(B)]).astype(np.float32)


LAST_RESULTS = None


# revision 5
# speedup vs baseline: 1.2054x; 1.2054x over previous
"""BiMPM kernel for Trainium2 — v4.

- Mixed 8-wide attentive-max groups: per group, lanes 0-1 are PE-fed
  (one-hot broadcast -> PSUM -> ACT mul), lanes 2-4 ACT-mul from a DMA
  broadcast load, lanes 5-7 DVE tensor_scalar mul; one 8-wide DVE
  tensor_tensor max per group into ping-pong accumulators.
- Broadcast rows come from host-replicated c1rep/c2rep in DRAM (every
  partition holds every row) so one grouped HWDGE dma_start per group
  runs near line rate.
- All input loads are emitted before any compute (big mm tensors on the
  scalar queue, the rest + all broadcasts on sync) so no DMA issue ever
  queues behind compute ops.
- mpm feature blocks are phase-split (matmul outputs staged to SBUF,
  ACT sqrt deferred) so the in-order ACT queue never stalls on them.

Self-contained: hardcodes B=8, L=128, H=768, P=16.
"""
import sys

sys.path.insert(0, "/opt/trn_rl_repo")

import numpy as np
import ml_dtypes
from contextlib import ExitStack

from concourse import bacc, mybir, masks
import concourse.tile as tile
from concourse.bass_utils import run_bass_kernel_spmd
from concourse.bass import MemorySpace

B, L, H, PP, NCH, NF = 8, 128, 768, 16, 6, 102
EPS = 1e-8
F32 = mybir.dt.float32
BF16 = mybir.dt.bfloat16
AX = mybir.AxisListType
OP = mybir.AluOpType
AF = mybir.ActivationFunctionType

BLK_ATT = slice(0, 17)    # w2ab columns: [ones|att16 | ones|matt16]
BLK_MATT = slice(17, 34)

G = 8        # j's per group
NPE = 2      # PE-fed lanes per group (ACT mul from PSUM)
NACT = 3     # DMA-fed ACT-mul lanes
# remaining lanes are DMA-fed DVE-mul lanes


class _AttMax:
    """Emits one side's attentive-max loop in resumable chunks."""

    def __init__(self, nc, pools, rep_dram, x_sb, identb, cosMcols, jmax,
                 tag, qsel):
        self.nc = nc
        self.sb, self.tbp, self.bcp, self.psbc = pools
        self.rep = rep_dram
        self.x_sb = x_sb
        self.identb = identb
        self.cols = cosMcols
        self.groups = [list(range(g, min(g + G, jmax)))
                       for g in range(0, jmax, G)]
        self.tag = tag
        self.qsel = qsel
        self.accs = []
        for k in range(2):
            acc = self.sb.tile([L, G, H], BF16, tag=f"{tag}_acc{k}")
            self.accs.append(acc)
        self.step = 0

    def emit(self, n):
        nc = self.nc
        while n > 0 and self.step < len(self.groups):
            g = self.step
            js = self.groups[g]
            cnt = len(js)
            npe = min(NPE, cnt)
            ndma = cnt - npe
            if ndma:
                bc6 = self.bcp.tile([L, G - NPE, H], BF16, tag="bc6")
                j0 = js[npe]
                nc.sync.dma_start(bc6[:, 0:ndma, :],
                                  self.rep[:, j0 * H:(j0 + ndma) * H])
            first = g == 0
            tb8 = self.accs[0] if first else self.tbp.tile(
                [L, G, H], BF16, tag="tb8")
            for t in range(npe):
                j = js[t]
                bc = self.psbc.tile([L, H], F32, tag="bc")
                sel = self.identb[:, j:j + 1].to_broadcast([L, L])
                nc.tensor.matmul(bc[:, 0:512], sel, self.x_sb[:, 0:512],
                                 start=True, stop=True)
                nc.tensor.matmul(bc[:, 512:H], sel, self.x_sb[:, 512:H],
                                 start=True, stop=True)
                nc.scalar.mul(tb8[:, t, :], bc[:], self.cols[:, j:j + 1])
            for t in range(npe, cnt):
                j = js[t]
                src = bc6[:, t - npe, :]
                if t < npe + NACT:
                    nc.scalar.mul(tb8[:, t, :], src, self.cols[:, j:j + 1])
                else:
                    nc.vector.tensor_scalar(
                        tb8[:, t, :], src, self.cols[:, j:j + 1], None,
                        OP.mult)
            if cnt < G:
                nc.vector.memset(tb8[:, cnt:G, :], -1e30)
            if not first:
                prev = self.accs[(self.step + 1) % 2]
                nxt = self.accs[self.step % 2]
                nc.vector.tensor_tensor(nxt[:], prev[:], tb8[:], op=OP.max)
            self.step += 1
            n -= 1
        return self.step >= len(self.groups)

    def fin_acc(self):
        return self.accs[(self.step + 1) % 2] if self.step > 1 else self.accs[0]

    def fold(self):
        nc = self.nc
        fin = self.fin_acc()
        h4 = self.sb.tile([L, 4, H], BF16, tag=self.tag + "_h4")
        nc.vector.tensor_tensor(h4[:], fin[:, 0:4, :], fin[:, 4:8, :],
                                op=OP.max)
        h2 = self.sb.tile([L, 2, H], BF16, tag=self.tag + "_h2")
        nc.vector.tensor_tensor(h2[:], h4[:, 0:2, :], h4[:, 2:4, :],
                                op=OP.max)
        out = self.sb.tile([L, H], BF16, tag=self.tag + "_o")
        nc.vector.tensor_tensor(out[:], h2[:, 0, :], h2[:, 1, :], op=OP.max)
        return out

    def fold_chunk(self, out, c):
        # per-128-column fold so the consumer can start before the full fold
        nc = self.nc
        fin = self.fin_acc()
        cs = slice(c * L, (c + 1) * L)
        h4 = self.sb.tile([L, 4, L], BF16, tag=self.tag + "_h4")
        nc.vector.tensor_tensor(h4[:], fin[:, 0:4, cs], fin[:, 4:8, cs],
                                op=OP.max)
        h2 = self.sb.tile([L, 2, L], BF16, tag=self.tag + "_h2")
        nc.vector.tensor_tensor(h2[:], h4[:, 0:2, :], h4[:, 2:4, :],
                                op=OP.max)
        nc.vector.tensor_tensor(out[:, cs], h2[:, 0, :], h2[:, 1, :],
                                op=OP.max)


def _trace_kernel(tc, dins, dout, jmax1, jmax2):
    nc = tc.nc
    with ExitStack() as ctx:
        sb = ctx.enter_context(tc.tile_pool(name="sb", bufs=1))
        sc = ctx.enter_context(tc.tile_pool(name="sc", bufs=3))
        tbp = ctx.enter_context(tc.tile_pool(name="tbp", bufs=2))
        bcp = ctx.enter_context(tc.tile_pool(name="bcp", bufs=4))
        ps_bc = ctx.enter_context(
            tc.tile_pool(name="ps_bc", bufs=2, space=MemorySpace.PSUM))
        ps_mm = ctx.enter_context(
            tc.tile_pool(name="ps_mm", bufs=1, space=MemorySpace.PSUM))
        ps_t = ctx.enter_context(
            tc.tile_pool(name="ps_t", bufs=1, space=MemorySpace.PSUM))
        ps_w = ctx.enter_context(
            tc.tile_pool(name="ps_w", bufs=1, space=MemorySpace.PSUM))

        def load(name, shape, dt=F32, rearr=None, eng=None, **kw):
            t = sb.tile(shape, dt, tag=name)
            src = dins[name][:]
            if rearr is not None:
                src = src.rearrange(rearr, **kw)
            (eng or nc.sync).dma_start(t[:], src)
            return t

        # ---- all input loads up front; nothing else on these queues yet ----
        c1T = load("c1T", [L, NCH, L], BF16, "(c p) n -> p c n", p=L)
        c2T = load("c2T", [L, NCH, L], BF16, "(c p) n -> p c n", p=L)
        rw1 = load("rw1", [L, 68])
        rw2 = load("rw2", [L, 68])
        invl1 = load("invl1", [L, 1])
        invl2 = load("invl2", [L, 1])
        mone1b = load("mone1b", [L, L])
        mone2b = load("mone2b", [L, L])
        c1b = load("c1b", [L, H], BF16)
        c2b = load("c2b", [L, H], BF16)
        c1x = load("c1x", [L, H], BF16)
        c2x = load("c2x", [L, H], BF16)
        rhs1 = load("rhs1", [L, NCH, 34], BF16, "(c p) n -> p c n", p=L)
        rhs2 = load("rhs2", [L, NCH, 34], BF16, "(c p) n -> p c n", p=L)
        w2ab = load("w2ab", [L, NCH, 34], BF16, "(c p) n -> p c n", p=L)
        rw1mp = load("rw1mp", [L, PP])
        rw2mp = load("rw2mp", [L, PP])
        # big, needed late: scalar HWDGE queue (no ACT compute emitted yet)
        mmr1 = load("mmr1", [L, NCH, PP * L], BF16, "(c p) n -> p c n", p=L,
                    eng=nc.scalar)
        mmr2 = load("mmr2", [L, NCH, PP * L], BF16, "(c p) n -> p c n", p=L,
                    eng=nc.scalar)
        mme1 = load("mme1", [L, NCH, PP], BF16, "(c p) n -> p c n", p=L,
                    eng=nc.scalar)
        mme2 = load("mme2", [L, NCH, PP], BF16, "(c p) n -> p c n", p=L,
                    eng=nc.scalar)

        ident = sb.tile([L, L], F32, tag="ident")
        masks.make_identity(nc, ident[:])
        identb = sb.tile([L, L], BF16, tag="identb")
        masks.make_identity(nc, identb[:])
        ones_col = sb.tile([L, 1], F32, tag="ones_col")
        nc.vector.memset(ones_col[:], 1.0)

        out1 = sb.tile([L, NF], F32, tag="out1")
        out2 = sb.tile([L, NF], F32, tag="out2")

        # ---- cos chain ----
        dotsp = ps_t.tile([L, L], F32, tag="t")
        for c in range(NCH):
            nc.tensor.matmul(dotsp[:], c1T[:, c, :], c2T[:, c, :],
                             start=(c == 0), stop=(c == NCH - 1))
        wS = sc.tile([L, L], F32, tag="wS")
        nc.vector.tensor_scalar(wS[:], dotsp[:], rw1[:, 0:1], None, OP.mult)
        wTp = ps_t.tile([L, L], F32, tag="t")
        nc.tensor.transpose(wTp[:], wS[:], ident[:])
        cosT = sb.tile([L, L], F32, tag="cosT")
        nc.vector.tensor_scalar(cosT[:], wTp[:], rw2[:, 0:1], None, OP.mult)
        cosp = ps_t.tile([L, L], F32, tag="t")
        nc.tensor.transpose(cosp[:], cosT[:], ident[:])
        cos = sb.tile([L, L], F32, tag="cos")
        nc.scalar.copy(cos[:], cosp[:])
        cosb = sb.tile([L, L], BF16, tag="cosb")
        nc.vector.tensor_copy(cosb[:], cosp[:])
        cosTb = sb.tile([L, L], BF16, tag="cosTb")
        nc.vector.tensor_copy(cosTb[:], cosT[:])

        # ---- cosM / cosMT (+1.0 in invalid columns) ----
        cosM = sb.tile([L, L], F32, tag="cosM")
        nc.vector.tensor_tensor(cosM[:], cosp[:], mone2b[:], op=OP.add)
        cosMT = sb.tile([L, L], F32, tag="cosMT")
        nc.vector.tensor_tensor(cosMT[:], cosT[:], mone1b[:], op=OP.add)

        # ---- attentive mean (softmax 1/sum cancels in cosine features) ----
        def att_mean(lhsT, rhs, tag):
            sp = ps_bc.tile([L, H], F32, tag="bc")
            nc.tensor.matmul(sp[:, 0:512], lhsT[:], rhs[:, 0:512],
                             start=True, stop=True)
            nc.tensor.matmul(sp[:, 512:H], lhsT[:], rhs[:, 512:H],
                             start=True, stop=True)
            am = sb.tile([L, H], BF16, tag=tag)
            nc.scalar.activation(am[:], sp[:], AF.Exp, scale=1.0)
            return am

        am2 = att_mean(cosTb, c2b, "am2")   # [i,H]
        am1 = att_mean(cosb, c1b, "am1")    # [j,H]

        # ---- cmax / cmean ----
        def cmaxmean(cosA, cosB, invl, out):
            nc.vector.reduce_max(out[:, 0:1], cosA[:], axis=AX.X)
            mp = ps_w.tile([L, 34], F32, tag="wa")
            nc.tensor.matmul(mp[:, 0:1], cosB[:], ones_col[:], start=True,
                             stop=True)
            nc.vector.tensor_scalar(out[:, 1:2], mp[:, 0:1], invl[:, 0:1],
                                    None, OP.mult)

        cmaxmean(cos, cosT, invl2, out1)
        cmaxmean(cosT, cos, invl1, out2)

        # ---- ff/bf matvec features ----
        def ff_feats(cT, rhs, rw, out):
            ffp = ps_w.tile([L, 34], F32, tag="wa")
            for c in range(NCH):
                nc.tensor.matmul(ffp[:], cT[:, c, :], rhs[:, c, :],
                                 start=(c == 0), stop=(c == NCH - 1))
            nc.vector.tensor_tensor(out[:, 2:36], ffp[:], rw[:, 0:34],
                                    op=OP.mult)

        ff_feats(c1T, rhs1, rw1, out1)
        ff_feats(c2T, rhs2, rw2, out2)

        # ---- attentive max loop emitters ----
        att1 = _AttMax(nc, (sb, tbp, bcp, ps_bc), dins["c2rep"][:], c2x,
                       identb, cosM, jmax2, "amx2", 0)
        att2 = _AttMax(nc, (sb, tbp, bcp, ps_bc), dins["c1rep"][:], c1x,
                       identb, cosMT, jmax1, "amx1", 1)

        # ---- phase-split mpm feature block ----
        def mpm_p1(v, cT, blk, tag):
            vsqT = sc.tile([L, NCH, L], BF16, tag="vsqT")
            prT = sc.tile([L, NCH, L], BF16, tag="prT")
            for c in range(NCH):
                tp = ps_t.tile([L, L], BF16, tag="t")
                nc.tensor.transpose(tp[:], v[:, c * L:(c + 1) * L], identb[:])
                nc.scalar.square(vsqT[:, c, :], tp[:])
                nc.vector.tensor_tensor(prT[:, c, :], cT[:, c, :], tp[:],
                                        op=OP.mult)
            numpt = ps_w.tile([L, 34], F32, tag="wa")
            wnpt = ps_w.tile([L, 34], F32, tag="wb")
            for c in range(NCH):
                nc.tensor.matmul(numpt[:, 0:17], prT[:, c, :],
                                 w2ab[:, c, blk],
                                 start=(c == 0), stop=(c == NCH - 1))
            for c in range(NCH):
                nc.tensor.matmul(wnpt[:, 0:17], vsqT[:, c, :],
                                 w2ab[:, c, blk],
                                 start=(c == 0), stop=(c == NCH - 1))
            st = sb.tile([L, 34], F32, tag=tag + "_st")
            nc.vector.tensor_copy(st[:, 0:17], numpt[:, 0:17])
            nc.vector.tensor_copy(st[:, 17:34], wnpt[:, 0:17])
            return st

        def mpm_p2(st, rw_side, blk, out, col0):
            rwv = sc.tile([L, 17], F32, tag="rwv")
            nc.scalar.sqrt(rwv[:], st[:, 17:34])
            nc.vector.tensor_scalar(rwv[:], rwv[:], EPS, None, OP.max)
            nc.vector.reciprocal(rwv[:], rwv[:])
            ft = sc.tile([L, 17], F32, tag="ft")
            nc.vector.tensor_tensor(ft[:], st[:, 0:17],
                                    rw_side[:, 34 + blk.start:34 + blk.stop],
                                    op=OP.mult)
            nc.vector.tensor_tensor(out[:, col0:col0 + 17], ft[:], rwv[:],
                                    op=OP.mult)

        # ---- mm (pairwise multi-perspective) twin-matmul block ----
        def mm_qtr(cT, mmr, rwmp, out, qtr):
            o = ps_mm.tile([L, 4, L], F32, tag="mm")
            qs = slice(qtr * 4 * L, (qtr + 1) * 4 * L)
            for c in range(NCH):
                nc.tensor.matmul(o[:], cT[:, c, :], mmr[:, c, qs],
                                 start=(c == 0), stop=(c == NCH - 1))
            mx = sc.tile([L, 4], F32, tag="mx4")
            nc.vector.reduce_max(mx[:, :, None], o[:], axis=AX.X)
            nc.vector.tensor_tensor(
                out[:, 36 + qtr * 4:36 + (qtr + 1) * 4], mx[:],
                rwmp[:, qtr * 4:(qtr + 1) * 4], op=OP.mult)

        def mm_mean(cT, mme, rwmp, out):
            mnt = ps_w.tile([L, 34], F32, tag="wa")
            for c in range(NCH):
                nc.tensor.matmul(mnt[:, 0:PP], cT[:, c, :], mme[:, c, :],
                                 start=(c == 0), stop=(c == NCH - 1))
            nc.vector.tensor_tensor(out[:, 52:68], mnt[:, 0:PP], rwmp[:],
                                    op=OP.mult)

        # ---- interleaved emission ----
        att1.emit(2)
        st_am2 = mpm_p1(am2, c1T, BLK_ATT, "am2")
        att1.emit(1)
        mpm_p2(st_am2, rw1, BLK_ATT, out1, 68)
        att1.emit(1)
        st_am1 = mpm_p1(am1, c2T, BLK_ATT, "am1")
        att1.emit(1)
        mpm_p2(st_am1, rw2, BLK_ATT, out2, 68)
        att1.emit(1)
        mm_qtr(c1T, mmr1, rw1mp, out1, 0)
        att1.emit(2)
        mm_qtr(c1T, mmr1, rw1mp, out1, 1)
        att1.emit(2)
        mm_qtr(c1T, mmr1, rw1mp, out1, 2)
        att1.emit(2)
        mm_qtr(c1T, mmr1, rw1mp, out1, 3)
        att1.emit(2)
        mm_mean(c1T, mme1, rw1mp, out1)
        att1.emit(99)

        att2.emit(2)
        amx2 = att1.fold()
        att2.emit(2)
        st_x2 = mpm_p1(amx2, c1T, BLK_MATT, "amx2")
        att2.emit(1)
        mpm_p2(st_x2, rw1, BLK_MATT, out1, 85)
        att2.emit(1)
        mm_qtr(c2T, mmr2, rw2mp, out2, 0)
        att2.emit(2)
        mm_qtr(c2T, mmr2, rw2mp, out2, 1)
        att2.emit(2)
        mm_qtr(c2T, mmr2, rw2mp, out2, 2)
        att2.emit(2)
        mm_qtr(c2T, mmr2, rw2mp, out2, 3)
        att2.emit(2)
        mm_mean(c2T, mme2, rw2mp, out2)
        att2.emit(99)

        # chunked fold of side 2 overlapped with its mpm consumers
        amx1 = sb.tile([L, H], BF16, tag="amx1_o")
        for c in range(NCH):
            att2.fold_chunk(amx1, c)
        st_x1 = mpm_p1(amx1, c2T, BLK_MATT, "amx1")
        mpm_p2(st_x1, rw2, BLK_MATT, out2, 85)

        # ---- store ----
        nc.sync.dma_start(dout[0:L, 0:85], out1[:, 0:85])
        nc.sync.dma_start(dout[0:L, 85:NF], out1[:, 85:NF])
        nc.sync.dma_start(dout[L:2 * L, 0:85], out2[:, 0:85])
        nc.sync.dma_start(dout[L:2 * L, 85:NF], out2[:, 85:NF])


_CACHED = {}


def _build(jmax1, jmax2):
    key = (jmax1, jmax2)
    if key in _CACHED:
        return _CACHED[key]
    nc = bacc.Bacc("TRN2", target_bir_lowering=False, debug=False,
                   enable_asserts=False)
    dins = {}
    for name, shape, dt in [
            ("c1b", [L, H], BF16), ("c2b", [L, H], BF16),
            ("c1x", [L, H], BF16), ("c2x", [L, H], BF16),
            ("c1T", [H, L], BF16), ("c2T", [H, L], BF16),
            ("c1rep", [L, L * H], BF16), ("c2rep", [L, L * H], BF16),
            ("rhs1", [H, 34], BF16), ("rhs2", [H, 34], BF16),
            ("w2ab", [H, 34], BF16),
            ("mmr1", [H, PP * L], BF16), ("mmr2", [H, PP * L], BF16),
            ("mme1", [H, PP], BF16), ("mme2", [H, PP], BF16),
            ("rw1", [L, 68], F32), ("rw2", [L, 68], F32),
            ("rw1mp", [L, PP], F32), ("rw2mp", [L, PP], F32),
            ("mone1b", [L, L], F32), ("mone2b", [L, L], F32),
            ("invl1", [L, 1], F32), ("invl2", [L, 1], F32)]:
        dins[name] = nc.dram_tensor(name, shape, dt, kind="ExternalInput")
    dout = nc.dram_tensor("out", [2 * L, NF], F32, kind="ExternalOutput")
    with tile.TileContext(nc) as tc:
        _trace_kernel(tc, dins, dout[:], jmax1, jmax2)
    nc.compile()
    _CACHED[key] = nc
    return nc


def _host_prep(c1raw, m1, c2raw, m2, w_ff, w_fb, w_mp, w_att, w_matt):
    asb = lambda a: np.ascontiguousarray(a, dtype=ml_dtypes.bfloat16)
    asf = lambda a: np.ascontiguousarray(a, dtype=np.float32)

    c1 = (c1raw * m1[:, None]).astype(np.float32)
    c2 = (c2raw * m2[:, None]).astype(np.float32)
    len1, len2 = float(m1.sum()), float(m2.sum())
    lp1, lp2 = max(int(len1) - 1, 0), max(int(len2) - 1, 0)

    def mpm_rhs(v, w):
        w2 = w * w
        rn = 1.0 / max(np.sqrt((v * v).sum()), EPS)
        wn = np.sqrt((w2 * (v * v)[None, :]).sum(1))
        rwn = 1.0 / np.maximum(wn, EPS)
        return np.concatenate(
            [(v * rn)[:, None], (w2 * v[None, :] * rwn[:, None]).T], 1)

    rhs1 = np.concatenate([mpm_rhs(c2[lp2], w_ff), mpm_rhs(c2[0], w_fb)], 1)
    rhs2 = np.concatenate([mpm_rhs(c1[lp1], w_ff), mpm_rhs(c1[0], w_fb)], 1)

    # rw tables [L, 68]: [ones|ff16 | ones|fb16 | ones|att16 | ones|matt16]
    ones_h = np.ones((1, H), np.float32)
    wsq = np.concatenate([ones_h, w_ff**2, ones_h, w_fb**2,
                          ones_h, w_att**2, ones_h, w_matt**2], 0)  # [68,H]
    def rw_of(c):
        wn = np.sqrt(wsq @ (c * c).T)           # [68, L]
        return (1.0 / np.maximum(wn, EPS)).T    # [L, 68]
    rw1, rw2 = rw_of(c1), rw_of(c2)

    # mp folding
    w2mp = (w_mp * w_mp).astype(np.float32)          # [P, H]
    def rwmp_of(c):
        wn = np.sqrt(w2mp @ (c * c).T)               # [P, L]
        return 1.0 / np.maximum(wn, EPS)             # [P, L]
    rw1mp_t, rw2mp_t = rwmp_of(c1), rwmp_of(c2)      # [P, L]
    # mmr1[h, (p, j)] = w2mp[p,h] * c2[j,h] * rw2mp[p,j]
    mmr1 = np.einsum("ph,jh,pj->hpj", w2mp, c2, rw2mp_t).reshape(H, PP * L)
    mmr2 = np.einsum("ph,ih,pi->hpi", w2mp, c1, rw1mp_t).reshape(H, PP * L)
    # mme1[h, p] = w2mp[p,h] * (sum_j c2[j,h] rw2mp[p,j]) / len2
    s2 = np.einsum("jh,pj->ph", c2, rw2mp_t)
    s1 = np.einsum("ih,pi->ph", c1, rw1mp_t)
    mme1 = (w2mp * s2 / max(len2, EPS)).T            # [H, P]
    mme2 = (w2mp * s1 / max(len1, EPS)).T

    w2ab = np.concatenate([ones_h, w_att**2, ones_h, w_matt**2], 0).T  # [H,34]

    # broadcast-replicated masked rows: every partition holds every row
    c1x = asb(c1 + ((m1 - 1) * 1e30)[:, None])
    c2x = asb(c2 + ((m2 - 1) * 1e30)[:, None])
    c1rep = np.ascontiguousarray(
        np.broadcast_to(c1x.reshape(-1)[None, :], (L, L * H)))
    c2rep = np.ascontiguousarray(
        np.broadcast_to(c2x.reshape(-1)[None, :], (L, L * H)))

    bc = lambda r: np.ascontiguousarray(
        np.broadcast_to(r[None, :], (L, L)), dtype=np.float32)
    return dict(
        c1b=asb(c1), c2b=asb(c2), c1x=c1x, c2x=c2x,
        c1rep=c1rep, c2rep=c2rep,
        c1T=asb(c1.T), c2T=asb(c2.T),
        rhs1=asb(rhs1), rhs2=asb(rhs2), w2ab=asb(w2ab),
        mmr1=asb(mmr1), mmr2=asb(mmr2), mme1=asb(mme1), mme2=asb(mme2),
        rw1=asf(rw1), rw2=asf(rw2),
        rw1mp=asf(rw1mp_t.T), rw2mp=asf(rw2mp_t.T),
        mone1b=bc(1 - m1), mone2b=bc(1 - m2),
        invl1=np.full((L, 1), 1.0 / max(len1, EPS), np.float32),
        invl2=np.full((L, 1), 1.0 / max(len2, EPS), np.float32),
    )


def kernel(context_1, mask_1, context_2, mask_2,
           w_ff, w_fb, w_mp, w_att, w_matt, **_unused):
    context_1 = np.asarray(context_1, dtype=np.float32)
    context_2 = np.asarray(context_2, dtype=np.float32)
    mask_1 = np.asarray(mask_1, dtype=np.float32)
    mask_2 = np.asarray(mask_2, dtype=np.float32)
    w_ff, w_fb = np.asarray(w_ff, np.float32), np.asarray(w_fb, np.float32)
    w_mp = np.asarray(w_mp, np.float32)
    w_att, w_matt = np.asarray(w_att, np.float32), np.asarray(w_matt, np.float32)
    assert context_1.shape == (B, L, H), context_1.shape

    jmax1 = int(mask_1.sum(1).max())
    jmax2 = int(mask_2.sum(1).max())
    nc = _build(jmax1, jmax2)
    in_maps = [
        _host_prep(context_1[b], mask_1[b], context_2[b], mask_2[b],
                   w_ff, w_fb, w_mp, w_att, w_matt)
        for b in range(B)
    ]
    res = run_bass_kernel_spmd(nc, in_maps, core_ids=list(range(B)))
    global LAST_RESULTS
    LAST_RESULTS = res
    return np.stack([res.results[b]["out"] for b in range(B)]).astype(np.float32)


LAST_RESULTS = None


# revision 13
# speedup vs baseline: 1.3007x; 1.0791x over previous
"""BiMPM kernel for Trainium2 — v4.

- Mixed 8-wide attentive-max groups: per group, lanes 0-1 are PE-fed
  (one-hot broadcast -> PSUM -> ACT mul), lanes 2-4 ACT-mul from a DMA
  broadcast load, lanes 5-7 DVE tensor_scalar mul; one 8-wide DVE
  tensor_tensor max per group into ping-pong accumulators.
- Broadcast rows come from host-replicated c1rep/c2rep in DRAM (every
  partition holds every row) so one grouped HWDGE dma_start per group
  runs near line rate.
- All input loads are emitted before any compute (big mm tensors on the
  scalar queue, the rest + all broadcasts on sync) so no DMA issue ever
  queues behind compute ops.
- mpm feature blocks are phase-split (matmul outputs staged to SBUF,
  ACT sqrt deferred) so the in-order ACT queue never stalls on them.

Self-contained: hardcodes B=8, L=128, H=768, P=16.
"""
import sys

sys.path.insert(0, "/opt/trn_rl_repo")

import numpy as np
import ml_dtypes
from contextlib import ExitStack

from concourse import bacc, mybir, masks
import concourse.tile as tile
from concourse.bass_utils import run_bass_kernel_spmd
from concourse.bass import MemorySpace

B, L, H, PP, NCH, NF = 8, 128, 768, 16, 6, 102
EPS = 1e-8
F32 = mybir.dt.float32
BF16 = mybir.dt.bfloat16
AX = mybir.AxisListType
OP = mybir.AluOpType
AF = mybir.ActivationFunctionType

BLK_ATT = slice(0, 17)    # w2ab columns: [ones|att16 | ones|matt16]
BLK_MATT = slice(17, 34)

G = 8        # j's per group
NPE = 2      # PE-fed lanes per group (ACT mul from PSUM)
NACT = 2     # DMA-fed ACT-mul lanes
# remaining lanes are DMA-fed DVE-mul lanes


class _AttMax:
    """Emits one side's attentive-max loop in resumable chunks."""

    def __init__(self, nc, pools, rep_dram, x_sb, identb, cosMcols, jmax,
                 tag, qsel):
        self.nc = nc
        self.sb, self.tbp, self.bcp, self.psbc = pools
        self.rep = rep_dram
        self.x_sb = x_sb
        self.identb = identb
        self.cols = cosMcols
        self.groups = [list(range(g, min(g + G, jmax)))
                       for g in range(0, jmax, G)]
        self.tag = tag
        self.qsel = qsel
        self.accs = []
        for k in range(2):
            acc = self.sb.tile([L, G, H], BF16, tag=f"{tag}_acc{k}")
            self.accs.append(acc)
        self.step = 0
        self.dma_step = 0
        self.pending = []

    def _npe(self, g):
        # first two groups are all-DMA so the PE queue stays clear for
        # the cos chain they depend on
        return 0 if g < 2 else min(NPE, len(self.groups[g]))

    def emit_dma(self, n):
        nc = self.nc
        while n > 0 and self.dma_step < len(self.groups):
            g = self.dma_step
            js = self.groups[g]
            npe = self._npe(g)
            ndma = len(js) - npe
            bc = self.bcp.tile([L, G, H], BF16, tag="bc")
            if ndma:
                j0 = js[npe]
                nc.sync.dma_start(bc[:, 0:ndma, :],
                                  self.rep[:, j0 * H:(j0 + ndma) * H])
            self.pending.append(bc)
            self.dma_step += 1
            n -= 1

    def emit(self, n):
        nc = self.nc
        while n > 0 and self.step < len(self.groups):
            g = self.step
            if not self.pending:
                self.emit_dma(1)
            bc6 = self.pending.pop(0)
            js = self.groups[g]
            cnt = len(js)
            npe = self._npe(g)
            first = g == 0
            tb8 = self.accs[0] if first else self.tbp.tile(
                [L, G, H], BF16, tag="tb8")
            for t in range(npe):
                j = js[t]
                bc = self.psbc.tile([L, H], F32, tag="bc")
                sel = self.identb[:, j:j + 1].to_broadcast([L, L])
                nc.tensor.matmul(bc[:, 0:512], sel, self.x_sb[:, 0:512],
                                 start=True, stop=True)
                nc.tensor.matmul(bc[:, 512:H], sel, self.x_sb[:, 512:H],
                                 start=True, stop=True)
                nc.scalar.mul(tb8[:, t, :], bc[:], self.cols[:, j:j + 1])
            nact = NACT + (NPE - npe)  # all-DMA groups keep the ACT share
            for t in range(npe, cnt):
                j = js[t]
                src = bc6[:, t - npe, :]
                if t < npe + nact:
                    nc.scalar.mul(tb8[:, t, :], src, self.cols[:, j:j + 1])
                else:
                    nc.vector.tensor_scalar(
                        tb8[:, t, :], src, self.cols[:, j:j + 1], None,
                        OP.mult)
            if cnt < G:
                nc.vector.memset(tb8[:, cnt:G, :], -1e30)
            if not first:
                prev = self.accs[(self.step + 1) % 2]
                nxt = self.accs[self.step % 2]
                nc.vector.tensor_tensor(nxt[:], prev[:], tb8[:], op=OP.max)
            self.step += 1
            n -= 1
        return self.step >= len(self.groups)

    def fin_acc(self):
        return self.accs[(self.step + 1) % 2] if self.step > 1 else self.accs[0]

    def fold(self):
        nc = self.nc
        fin = self.fin_acc()
        h4 = self.sb.tile([L, 4, H], BF16, tag=self.tag + "_h4")
        nc.vector.tensor_tensor(h4[:], fin[:, 0:4, :], fin[:, 4:8, :],
                                op=OP.max)
        h2 = self.sb.tile([L, 2, H], BF16, tag=self.tag + "_h2")
        nc.vector.tensor_tensor(h2[:], h4[:, 0:2, :], h4[:, 2:4, :],
                                op=OP.max)
        out = self.sb.tile([L, H], BF16, tag=self.tag + "_o")
        nc.vector.tensor_tensor(out[:], h2[:, 0, :], h2[:, 1, :], op=OP.max)
        return out

    def fold_chunk(self, out, c):
        # per-128-column fold so the consumer can start before the full fold
        nc = self.nc
        fin = self.fin_acc()
        cs = slice(c * L, (c + 1) * L)
        h4 = self.sb.tile([L, 4, L], BF16, tag=self.tag + "_h4")
        nc.vector.tensor_tensor(h4[:], fin[:, 0:4, cs], fin[:, 4:8, cs],
                                op=OP.max)
        h2 = self.sb.tile([L, 2, L], BF16, tag=self.tag + "_h2")
        nc.vector.tensor_tensor(h2[:], h4[:, 0:2, :], h4[:, 2:4, :],
                                op=OP.max)
        nc.vector.tensor_tensor(out[:, cs], h2[:, 0, :], h2[:, 1, :],
                                op=OP.max)


def _trace_kernel(tc, dins, dout, jmax1, jmax2):
    nc = tc.nc
    with ExitStack() as ctx:
        sb = ctx.enter_context(tc.tile_pool(name="sb", bufs=1))
        sc = ctx.enter_context(tc.tile_pool(name="sc", bufs=3))
        tbp = ctx.enter_context(tc.tile_pool(name="tbp", bufs=2))
        bcp = ctx.enter_context(tc.tile_pool(name="bcp", bufs=3))
        ps_bc = ctx.enter_context(
            tc.tile_pool(name="ps_bc", bufs=2, space=MemorySpace.PSUM))
        ps_mm = ctx.enter_context(
            tc.tile_pool(name="ps_mm", bufs=1, space=MemorySpace.PSUM))
        ps_t = ctx.enter_context(
            tc.tile_pool(name="ps_t", bufs=1, space=MemorySpace.PSUM))
        ps_w = ctx.enter_context(
            tc.tile_pool(name="ps_w", bufs=1, space=MemorySpace.PSUM))

        def load(name, shape, dt=F32, rearr=None, eng=None, t=None, **kw):
            if t is None:
                t = sb.tile(shape, dt, tag=name)
            src = dins[name][:]
            if rearr is not None:
                src = src.rearrange(rearr, **kw)
            (eng or nc.sync).dma_start(t[:], src)
            return t

        # tiles the att emitters reference (filled later)
        c1x = sb.tile([L, H], BF16, tag="c1x")
        c2x = sb.tile([L, H], BF16, tag="c2x")
        identb = sb.tile([L, L], BF16, tag="identb")
        cosM = sb.tile([L, L], F32, tag="cosM")
        cosMT = sb.tile([L, L], F32, tag="cosMT")

        att1 = _AttMax(nc, (sb, tbp, bcp, ps_bc), dins["c2rep"][:], c2x,
                       identb, cosM, jmax2, "amx2", 0)
        att2 = _AttMax(nc, (sb, tbp, bcp, ps_bc), dins["c1rep"][:], c1x,
                       identb, cosMT, jmax1, "amx1", 1)
        # broadcast data for the first side-1 groups lands first
        att1.emit_dma(2)

        # ---- input loads; sync queue only carries loads + broadcasts ----
        c1T = load("c1T", [L, NCH, L], BF16, "(c p) n -> p c n", p=L)
        c2T = load("c2T", [L, NCH, L], BF16, "(c p) n -> p c n", p=L)
        rw1 = load("rw1", [L, 68])
        rw2 = load("rw2", [L, 68])
        load("c2x", None, t=c2x)
        load("c1x", None, t=c1x)
        invl1 = load("invl1", [L, 1])
        invl2 = load("invl2", [L, 1])
        mone1b = load("mone1b", [L, L])
        mone2b = load("mone2b", [L, L])
        c1b = load("c1b", [L, H], BF16)
        c2b = load("c2b", [L, H], BF16)
        rhs1 = load("rhs1", [L, NCH, 34], BF16, "(c p) n -> p c n", p=L)
        rhs2 = load("rhs2", [L, NCH, 34], BF16, "(c p) n -> p c n", p=L)
        w2ab = load("w2ab", [L, NCH, 34], BF16, "(c p) n -> p c n", p=L)
        rw1mp = load("rw1mp", [L, PP])
        rw2mp = load("rw2mp", [L, PP])
        mme1 = load("mme1", [L, NCH, PP], BF16, "(c p) n -> p c n", p=L)
        mme2 = load("mme2", [L, NCH, PP], BF16, "(c p) n -> p c n", p=L)

        # mm twin-matmul rhs, loaded per quarter just-in-time
        def load_mmr_qtr(name, qtr):
            t = sb.tile([L, NCH, 4 * L], BF16, tag=f"{name}q{qtr}")
            src = dins[name][:].rearrange("(c p) n -> p c n", p=L)
            qs = slice(qtr * 4 * L, (qtr + 1) * 4 * L)
            nc.sync.dma_start(t[:], src[:, :, qs])
            return t

        ident = sb.tile([L, L], F32, tag="ident")
        masks.make_identity(nc, ident[:])
        masks.make_identity(nc, identb[:])
        ones_col = sb.tile([L, 1], F32, tag="ones_col")
        nc.vector.memset(ones_col[:], 1.0)

        out1 = sb.tile([L, NF], F32, tag="out1")
        out2 = sb.tile([L, NF], F32, tag="out2")

        # ---- cos chain ----
        dotsp = ps_t.tile([L, L], F32, tag="t")
        for c in range(NCH):
            nc.tensor.matmul(dotsp[:], c1T[:, c, :], c2T[:, c, :],
                             start=(c == 0), stop=(c == NCH - 1))
        wS = sc.tile([L, L], F32, tag="wS")
        nc.vector.tensor_scalar(wS[:], dotsp[:], rw1[:, 0:1], None, OP.mult)
        wTp = ps_t.tile([L, L], F32, tag="t")
        nc.tensor.transpose(wTp[:], wS[:], ident[:])
        cosT = sb.tile([L, L], F32, tag="cosT")
        nc.vector.tensor_scalar(cosT[:], wTp[:], rw2[:, 0:1], None, OP.mult)
        cosp = ps_t.tile([L, L], F32, tag="t")
        nc.tensor.transpose(cosp[:], cosT[:], ident[:])
        cos = sb.tile([L, L], F32, tag="cos")
        nc.scalar.copy(cos[:], cosp[:])
        cosb = sb.tile([L, L], BF16, tag="cosb")
        nc.vector.tensor_copy(cosb[:], cosp[:])
        cosTb = sb.tile([L, L], BF16, tag="cosTb")
        nc.vector.tensor_copy(cosTb[:], cosT[:])

        # ---- cosM / cosMT (+1.0 in invalid columns) ----
        nc.vector.tensor_tensor(cosM[:], cosp[:], mone2b[:], op=OP.add)
        nc.vector.tensor_tensor(cosMT[:], cosT[:], mone1b[:], op=OP.add)

        # ---- attentive mean (softmax 1/sum cancels in cosine features) ----
        def att_mean(lhsT, rhs, tag):
            sp = ps_bc.tile([L, H], F32, tag="bc")
            nc.tensor.matmul(sp[:, 0:512], lhsT[:], rhs[:, 0:512],
                             start=True, stop=True)
            nc.tensor.matmul(sp[:, 512:H], lhsT[:], rhs[:, 512:H],
                             start=True, stop=True)
            am = sb.tile([L, H], BF16, tag=tag)
            nc.scalar.activation(am[:], sp[:], AF.Exp, scale=1.0)
            return am

        am2 = att_mean(cosTb, c2b, "am2")   # [i,H]
        am1 = att_mean(cosb, c1b, "am1")    # [j,H]

        # ---- cmax / cmean ----
        def cmaxmean(cosA, cosB, invl, out):
            nc.vector.reduce_max(out[:, 0:1], cosA[:], axis=AX.X)
            mp = ps_w.tile([L, 34], F32, tag="wa")
            nc.tensor.matmul(mp[:, 0:1], cosB[:], ones_col[:], start=True,
                             stop=True)
            nc.vector.tensor_scalar(out[:, 1:2], mp[:, 0:1], invl[:, 0:1],
                                    None, OP.mult)

        cmaxmean(cos, cosT, invl2, out1)
        cmaxmean(cosT, cos, invl1, out2)

        # ---- ff/bf matvec features ----
        def ff_feats(cT, rhs, rw, out):
            ffp = ps_w.tile([L, 34], F32, tag="wa")
            for c in range(NCH):
                nc.tensor.matmul(ffp[:], cT[:, c, :], rhs[:, c, :],
                                 start=(c == 0), stop=(c == NCH - 1))
            nc.vector.tensor_tensor(out[:, 2:36], ffp[:], rw[:, 0:34],
                                    op=OP.mult)

        ff_feats(c1T, rhs1, rw1, out1)
        ff_feats(c2T, rhs2, rw2, out2)

        # ---- phase-split mpm feature block ----
        def mpm_p1(v, cT, blk, tag):
            vsqT = sc.tile([L, NCH, L], BF16, tag="vsqT")
            prT = sc.tile([L, NCH, L], BF16, tag="prT")
            for c in range(NCH):
                tp = ps_t.tile([L, L], BF16, tag="t")
                nc.tensor.transpose(tp[:], v[:, c * L:(c + 1) * L], identb[:])
                nc.scalar.square(vsqT[:, c, :], tp[:])
                nc.vector.tensor_tensor(prT[:, c, :], cT[:, c, :], tp[:],
                                        op=OP.mult)
            numpt = ps_w.tile([L, 34], F32, tag="wa")
            wnpt = ps_w.tile([L, 34], F32, tag="wb")
            for c in range(NCH):
                nc.tensor.matmul(numpt[:, 0:17], prT[:, c, :],
                                 w2ab[:, c, blk],
                                 start=(c == 0), stop=(c == NCH - 1))
            for c in range(NCH):
                nc.tensor.matmul(wnpt[:, 0:17], vsqT[:, c, :],
                                 w2ab[:, c, blk],
                                 start=(c == 0), stop=(c == NCH - 1))
            st = sb.tile([L, 34], F32, tag=tag + "_st")
            nc.vector.tensor_copy(st[:, 0:17], numpt[:, 0:17])
            nc.vector.tensor_copy(st[:, 17:34], wnpt[:, 0:17])
            return st

        def mpm_p2(st, rw_side, blk, out, col0):
            rwv = sc.tile([L, 17], F32, tag="rwv")
            nc.scalar.sqrt(rwv[:], st[:, 17:34])
            nc.vector.tensor_scalar(rwv[:], rwv[:], EPS, None, OP.max)
            nc.vector.reciprocal(rwv[:], rwv[:])
            ft = sc.tile([L, 17], F32, tag="ft")
            nc.vector.tensor_tensor(ft[:], st[:, 0:17],
                                    rw_side[:, 34 + blk.start:34 + blk.stop],
                                    op=OP.mult)
            nc.vector.tensor_tensor(out[:, col0:col0 + 17], ft[:], rwv[:],
                                    op=OP.mult)

        # ---- mm (pairwise multi-perspective) twin-matmul block ----
        def mm_qtr(cT, mmrq, rwmp, out, qtr):
            o = ps_mm.tile([L, 4, L], F32, tag="mm")
            for c in range(NCH):
                nc.tensor.matmul(o[:], cT[:, c, :], mmrq[:, c, :],
                                 start=(c == 0), stop=(c == NCH - 1))
            mx = sc.tile([L, 4], F32, tag="mx4")
            nc.vector.reduce_max(mx[:, :, None], o[:], axis=AX.X)
            nc.vector.tensor_tensor(
                out[:, 36 + qtr * 4:36 + (qtr + 1) * 4], mx[:],
                rwmp[:, qtr * 4:(qtr + 1) * 4], op=OP.mult)

        def mm_mean(cT, mme, rwmp, out):
            mnt = ps_w.tile([L, 34], F32, tag="wa")
            for c in range(NCH):
                nc.tensor.matmul(mnt[:, 0:PP], cT[:, c, :], mme[:, c, :],
                                 start=(c == 0), stop=(c == NCH - 1))
            nc.vector.tensor_tensor(out[:, 52:68], mnt[:, 0:PP], rwmp[:],
                                    op=OP.mult)

        # ---- interleaved emission ----
        att1.emit(2)
        st_am2 = mpm_p1(am2, c1T, BLK_ATT, "am2")
        att1.emit(1)
        mpm_p2(st_am2, rw1, BLK_ATT, out1, 68)
        att1.emit(1)
        st_am1 = mpm_p1(am1, c2T, BLK_ATT, "am1")
        att1.emit(1)
        mr10 = load_mmr_qtr("mmr1", 0)
        mpm_p2(st_am1, rw2, BLK_ATT, out2, 68)
        att1.emit(1)
        mr11 = load_mmr_qtr("mmr1", 1)
        mm_qtr(c1T, mr10, rw1mp, out1, 0)
        att1.emit(2)
        mr12 = load_mmr_qtr("mmr1", 2)
        mm_qtr(c1T, mr11, rw1mp, out1, 1)
        att1.emit(2)
        mr13 = load_mmr_qtr("mmr1", 3)
        mm_qtr(c1T, mr12, rw1mp, out1, 2)
        att1.emit(2)
        mm_qtr(c1T, mr13, rw1mp, out1, 3)
        att1.emit(2)
        mm_mean(c1T, mme1, rw1mp, out1)
        att1.emit(99)

        att2.emit(2)
        amx2 = att1.fold()
        att2.emit(2)
        st_x2 = mpm_p1(amx2, c1T, BLK_MATT, "amx2")
        att2.emit(1)
        mr20 = load_mmr_qtr("mmr2", 0)
        mpm_p2(st_x2, rw1, BLK_MATT, out1, 85)
        att2.emit(1)
        mr21 = load_mmr_qtr("mmr2", 1)
        mm_qtr(c2T, mr20, rw2mp, out2, 0)
        att2.emit(2)
        mr22 = load_mmr_qtr("mmr2", 2)
        mm_qtr(c2T, mr21, rw2mp, out2, 1)
        att2.emit(2)
        mr23 = load_mmr_qtr("mmr2", 3)
        mm_qtr(c2T, mr22, rw2mp, out2, 2)
        att2.emit(2)
        mm_qtr(c2T, mr23, rw2mp, out2, 3)
        att2.emit(2)
        mm_mean(c2T, mme2, rw2mp, out2)
        att2.emit(99)

        # chunked fold of side 2 overlapped with its mpm consumers
        amx1 = sb.tile([L, H], BF16, tag="amx1_o")
        for c in range(NCH):
            att2.fold_chunk(amx1, c)
        st_x1 = mpm_p1(amx1, c2T, BLK_MATT, "amx1")
        mpm_p2(st_x1, rw2, BLK_MATT, out2, 85)

        # ---- store ----
        nc.sync.dma_start(dout[0:L, 0:85], out1[:, 0:85])
        nc.sync.dma_start(dout[0:L, 85:NF], out1[:, 85:NF])
        nc.sync.dma_start(dout[L:2 * L, 0:85], out2[:, 0:85])
        nc.sync.dma_start(dout[L:2 * L, 85:NF], out2[:, 85:NF])


_CACHED = {}


def _build(jmax1, jmax2):
    key = (jmax1, jmax2)
    if key in _CACHED:
        return _CACHED[key]
    nc = bacc.Bacc("TRN2", target_bir_lowering=False, debug=False,
                   enable_asserts=False)
    dins = {}
    for name, shape, dt in [
            ("c1b", [L, H], BF16), ("c2b", [L, H], BF16),
            ("c1x", [L, H], BF16), ("c2x", [L, H], BF16),
            ("c1T", [H, L], BF16), ("c2T", [H, L], BF16),
            ("c1rep", [L, L * H], BF16), ("c2rep", [L, L * H], BF16),
            ("rhs1", [H, 34], BF16), ("rhs2", [H, 34], BF16),
            ("w2ab", [H, 34], BF16),
            ("mmr1", [H, PP * L], BF16), ("mmr2", [H, PP * L], BF16),
            ("mme1", [H, PP], BF16), ("mme2", [H, PP], BF16),
            ("rw1", [L, 68], F32), ("rw2", [L, 68], F32),
            ("rw1mp", [L, PP], F32), ("rw2mp", [L, PP], F32),
            ("mone1b", [L, L], F32), ("mone2b", [L, L], F32),
            ("invl1", [L, 1], F32), ("invl2", [L, 1], F32)]:
        dins[name] = nc.dram_tensor(name, shape, dt, kind="ExternalInput")
    dout = nc.dram_tensor("out", [2 * L, NF], F32, kind="ExternalOutput")
    with tile.TileContext(nc) as tc:
        _trace_kernel(tc, dins, dout[:], jmax1, jmax2)
    nc.compile()
    _CACHED[key] = nc
    return nc


def _host_prep(c1raw, m1, c2raw, m2, w_ff, w_fb, w_mp, w_att, w_matt):
    asb = lambda a: np.ascontiguousarray(a, dtype=ml_dtypes.bfloat16)
    asf = lambda a: np.ascontiguousarray(a, dtype=np.float32)

    c1 = (c1raw * m1[:, None]).astype(np.float32)
    c2 = (c2raw * m2[:, None]).astype(np.float32)
    len1, len2 = float(m1.sum()), float(m2.sum())
    lp1, lp2 = max(int(len1) - 1, 0), max(int(len2) - 1, 0)

    def mpm_rhs(v, w):
        w2 = w * w
        rn = 1.0 / max(np.sqrt((v * v).sum()), EPS)
        wn = np.sqrt((w2 * (v * v)[None, :]).sum(1))
        rwn = 1.0 / np.maximum(wn, EPS)
        return np.concatenate(
            [(v * rn)[:, None], (w2 * v[None, :] * rwn[:, None]).T], 1)

    rhs1 = np.concatenate([mpm_rhs(c2[lp2], w_ff), mpm_rhs(c2[0], w_fb)], 1)
    rhs2 = np.concatenate([mpm_rhs(c1[lp1], w_ff), mpm_rhs(c1[0], w_fb)], 1)

    # rw tables [L, 68]: [ones|ff16 | ones|fb16 | ones|att16 | ones|matt16]
    ones_h = np.ones((1, H), np.float32)
    wsq = np.concatenate([ones_h, w_ff**2, ones_h, w_fb**2,
                          ones_h, w_att**2, ones_h, w_matt**2], 0)  # [68,H]
    def rw_of(c):
        wn = np.sqrt(wsq @ (c * c).T)           # [68, L]
        return (1.0 / np.maximum(wn, EPS)).T    # [L, 68]
    rw1, rw2 = rw_of(c1), rw_of(c2)

    # mp folding
    w2mp = (w_mp * w_mp).astype(np.float32)          # [P, H]
    def rwmp_of(c):
        wn = np.sqrt(w2mp @ (c * c).T)               # [P, L]
        return 1.0 / np.maximum(wn, EPS)             # [P, L]
    rw1mp_t, rw2mp_t = rwmp_of(c1), rwmp_of(c2)      # [P, L]
    # mmr1[h, (p, j)] = w2mp[p,h] * c2[j,h] * rw2mp[p,j]
    mmr1 = np.einsum("ph,jh,pj->hpj", w2mp, c2, rw2mp_t).reshape(H, PP * L)
    mmr2 = np.einsum("ph,ih,pi->hpi", w2mp, c1, rw1mp_t).reshape(H, PP * L)
    # mme1[h, p] = w2mp[p,h] * (sum_j c2[j,h] rw2mp[p,j]) / len2
    s2 = np.einsum("jh,pj->ph", c2, rw2mp_t)
    s1 = np.einsum("ih,pi->ph", c1, rw1mp_t)
    mme1 = (w2mp * s2 / max(len2, EPS)).T            # [H, P]
    mme2 = (w2mp * s1 / max(len1, EPS)).T

    w2ab = np.concatenate([ones_h, w_att**2, ones_h, w_matt**2], 0).T  # [H,34]

    # broadcast-replicated masked rows: every partition holds every row
    c1x = asb(c1 + ((m1 - 1) * 1e30)[:, None])
    c2x = asb(c2 + ((m2 - 1) * 1e30)[:, None])
    c1rep = np.ascontiguousarray(
        np.broadcast_to(c1x.reshape(-1)[None, :], (L, L * H)))
    c2rep = np.ascontiguousarray(
        np.broadcast_to(c2x.reshape(-1)[None, :], (L, L * H)))

    bc = lambda r: np.ascontiguousarray(
        np.broadcast_to(r[None, :], (L, L)), dtype=np.float32)
    return dict(
        c1b=asb(c1), c2b=asb(c2), c1x=c1x, c2x=c2x,
        c1rep=c1rep, c2rep=c2rep,
        c1T=asb(c1.T), c2T=asb(c2.T),
        rhs1=asb(rhs1), rhs2=asb(rhs2), w2ab=asb(w2ab),
        mmr1=asb(mmr1), mmr2=asb(mmr2), mme1=asb(mme1), mme2=asb(mme2),
        rw1=asf(rw1), rw2=asf(rw2),
        rw1mp=asf(rw1mp_t.T), rw2mp=asf(rw2mp_t.T),
        mone1b=bc(1 - m1), mone2b=bc(1 - m2),
        invl1=np.full((L, 1), 1.0 / max(len1, EPS), np.float32),
        invl2=np.full((L, 1), 1.0 / max(len2, EPS), np.float32),
    )


def kernel(context_1, mask_1, context_2, mask_2,
           w_ff, w_fb, w_mp, w_att, w_matt, **_unused):
    context_1 = np.asarray(context_1, dtype=np.float32)
    context_2 = np.asarray(context_2, dtype=np.float32)
    mask_1 = np.asarray(mask_1, dtype=np.float32)
    mask_2 = np.asarray(mask_2, dtype=np.float32)
    w_ff, w_fb = np.asarray(w_ff, np.float32), np.asarray(w_fb, np.float32)
    w_mp = np.asarray(w_mp, np.float32)
    w_att, w_matt = np.asarray(w_att, np.float32), np.asarray(w_matt, np.float32)
    assert context_1.shape == (B, L, H), context_1.shape

    jmax1 = int(mask_1.sum(1).max())
    jmax2 = int(mask_2.sum(1).max())
    nc = _build(jmax1, jmax2)
    in_maps = [
        _host_prep(context_1[b], mask_1[b], context_2[b], mask_2[b],
                   w_ff, w_fb, w_mp, w_att, w_matt)
        for b in range(B)
    ]
    res = run_bass_kernel_spmd(nc, in_maps, core_ids=list(range(B)))
    global LAST_RESULTS
    LAST_RESULTS = res
    return np.stack([res.results[b]["out"] for b in range(B)]).astype(np.float32)


LAST_RESULTS = None


# revision 15
# speedup vs baseline: 1.3517x; 1.0392x over previous
"""BiMPM kernel for Trainium2 — v4.

- Mixed 8-wide attentive-max groups: per group, lanes 0-1 are PE-fed
  (one-hot broadcast -> PSUM -> ACT mul), lanes 2-4 ACT-mul from a DMA
  broadcast load, lanes 5-7 DVE tensor_scalar mul; one 8-wide DVE
  tensor_tensor max per group into ping-pong accumulators.
- Broadcast rows come from host-replicated c1rep/c2rep in DRAM (every
  partition holds every row) so one grouped HWDGE dma_start per group
  runs near line rate.
- All input loads are emitted before any compute (big mm tensors on the
  scalar queue, the rest + all broadcasts on sync) so no DMA issue ever
  queues behind compute ops.
- mpm feature blocks are phase-split (matmul outputs staged to SBUF,
  ACT sqrt deferred) so the in-order ACT queue never stalls on them.

Self-contained: hardcodes B=8, L=128, H=768, P=16.
"""
import sys

sys.path.insert(0, "/opt/trn_rl_repo")

import numpy as np
import ml_dtypes
from contextlib import ExitStack

from concourse import bacc, mybir, masks
import concourse.tile as tile
from concourse.bass_utils import run_bass_kernel_spmd
from concourse.bass import MemorySpace

B, L, H, PP, NCH, NF = 8, 128, 768, 16, 6, 102
EPS = 1e-8
F32 = mybir.dt.float32
BF16 = mybir.dt.bfloat16
AX = mybir.AxisListType
OP = mybir.AluOpType
AF = mybir.ActivationFunctionType

BLK_ATT = slice(0, 17)    # w2ab columns: [ones|att16 | ones|matt16]
BLK_MATT = slice(17, 34)

G = 8        # j's per group
NPE = 2      # PE-fed lanes per group (ACT mul from PSUM)
NACT = 2     # DMA-fed ACT-mul lanes
# remaining lanes are DMA-fed DVE-mul lanes


class _AttMax:
    """Emits one side's attentive-max loop in resumable chunks."""

    def __init__(self, nc, pools, rep_dram, x_sb, identb, cosMcols, jmax,
                 tag, qsel):
        self.nc = nc
        self.sb, self.tbp, self.bcp, self.psbc = pools
        self.rep = rep_dram
        self.x_sb = x_sb
        self.identb = identb
        self.cols = cosMcols
        self.groups = [list(range(g, min(g + G, jmax)))
                       for g in range(0, jmax, G)]
        self.tag = tag
        self.qsel = qsel
        self.accs = []
        for k in range(2):
            acc = self.sb.tile([L, G, H], BF16, tag=f"{tag}_acc{k}")
            self.accs.append(acc)
        self.step = 0
        self.dma_step = 0
        self.pending = []

    def _npe(self, g):
        # first two groups are all-DMA so the PE queue stays clear for
        # the cos chain they depend on
        return 0 if g < 2 else min(NPE, len(self.groups[g]))

    def emit_dma(self, n):
        nc = self.nc
        while n > 0 and self.dma_step < len(self.groups):
            g = self.dma_step
            js = self.groups[g]
            npe = self._npe(g)
            ndma = len(js) - npe
            bc = self.bcp.tile([L, G, H], BF16, tag="bc")
            if ndma:
                j0 = js[npe]
                nc.sync.dma_start(bc[:, 0:ndma, :],
                                  self.rep[:, j0 * H:(j0 + ndma) * H])
            self.pending.append(bc)
            self.dma_step += 1
            n -= 1

    def emit(self, n):
        nc = self.nc
        while n > 0 and self.step < len(self.groups):
            g = self.step
            if not self.pending:
                self.emit_dma(1)
            bc6 = self.pending.pop(0)
            js = self.groups[g]
            cnt = len(js)
            npe = self._npe(g)
            first = g == 0
            tb8 = self.accs[0] if first else self.tbp.tile(
                [L, G, H], BF16, tag="tb8")
            for t in range(npe):
                j = js[t]
                bc = self.psbc.tile([L, H], F32, tag="bc")
                sel = self.identb[:, j:j + 1].to_broadcast([L, L])
                nc.tensor.matmul(bc[:, 0:512], sel, self.x_sb[:, 0:512],
                                 start=True, stop=True)
                nc.tensor.matmul(bc[:, 512:H], sel, self.x_sb[:, 512:H],
                                 start=True, stop=True)
                nc.scalar.mul(tb8[:, t, :], bc[:], self.cols[:, j:j + 1])
            # ~2.75 of the 6 DMA lanes multiply on ACT, rest on DVE
            nact = (3 if g % 4 else 2) + (NPE - npe)
            for t in range(npe, G):
                tj = t if t < cnt else cnt - 1   # pad lanes repeat the last j
                j = js[tj]
                src = bc6[:, tj - npe, :]
                if t < npe + nact:
                    nc.scalar.mul(tb8[:, t, :], src, self.cols[:, j:j + 1])
                else:
                    nc.vector.tensor_scalar(
                        tb8[:, t, :], src, self.cols[:, j:j + 1], None,
                        OP.mult)
            if not first:
                prev = self.accs[(self.step + 1) % 2]
                nxt = self.accs[self.step % 2]
                nc.vector.tensor_tensor(nxt[:], prev[:], tb8[:], op=OP.max)
            self.step += 1
            n -= 1
        return self.step >= len(self.groups)

    def fin_acc(self):
        return self.accs[(self.step + 1) % 2] if self.step > 1 else self.accs[0]

    def fold(self):
        nc = self.nc
        fin = self.fin_acc()
        h4 = self.sb.tile([L, 4, H], BF16, tag=self.tag + "_h4")
        nc.vector.tensor_tensor(h4[:], fin[:, 0:4, :], fin[:, 4:8, :],
                                op=OP.max)
        h2 = self.sb.tile([L, 2, H], BF16, tag=self.tag + "_h2")
        nc.vector.tensor_tensor(h2[:], h4[:, 0:2, :], h4[:, 2:4, :],
                                op=OP.max)
        out = self.sb.tile([L, H], BF16, tag=self.tag + "_o")
        nc.vector.tensor_tensor(out[:], h2[:, 0, :], h2[:, 1, :], op=OP.max)
        return out

    def fold_chunk(self, out, c):
        # per-128-column fold so the consumer can start before the full fold
        nc = self.nc
        fin = self.fin_acc()
        cs = slice(c * L, (c + 1) * L)
        h4 = self.sb.tile([L, 4, L], BF16, tag=self.tag + "_h4")
        nc.vector.tensor_tensor(h4[:], fin[:, 0:4, cs], fin[:, 4:8, cs],
                                op=OP.max)
        h2 = self.sb.tile([L, 2, L], BF16, tag=self.tag + "_h2")
        nc.vector.tensor_tensor(h2[:], h4[:, 0:2, :], h4[:, 2:4, :],
                                op=OP.max)
        nc.vector.tensor_tensor(out[:, cs], h2[:, 0, :], h2[:, 1, :],
                                op=OP.max)


def _trace_kernel(tc, dins, dout, jmax1, jmax2):
    nc = tc.nc
    with ExitStack() as ctx:
        sb = ctx.enter_context(tc.tile_pool(name="sb", bufs=1))
        sc = ctx.enter_context(tc.tile_pool(name="sc", bufs=3))
        tbp = ctx.enter_context(tc.tile_pool(name="tbp", bufs=2))
        bcp = ctx.enter_context(tc.tile_pool(name="bcp", bufs=3))
        ps_bc = ctx.enter_context(
            tc.tile_pool(name="ps_bc", bufs=2, space=MemorySpace.PSUM))
        ps_mm = ctx.enter_context(
            tc.tile_pool(name="ps_mm", bufs=1, space=MemorySpace.PSUM))
        ps_t = ctx.enter_context(
            tc.tile_pool(name="ps_t", bufs=1, space=MemorySpace.PSUM))
        ps_w = ctx.enter_context(
            tc.tile_pool(name="ps_w", bufs=1, space=MemorySpace.PSUM))

        def load(name, shape, dt=F32, rearr=None, eng=None, t=None, **kw):
            if t is None:
                t = sb.tile(shape, dt, tag=name)
            src = dins[name][:]
            if rearr is not None:
                src = src.rearrange(rearr, **kw)
            (eng or nc.sync).dma_start(t[:], src)
            return t

        # tiles the att emitters reference (filled later)
        c1x = sb.tile([L, H], BF16, tag="c1x")
        c2x = sb.tile([L, H], BF16, tag="c2x")
        identb = sb.tile([L, L], BF16, tag="identb")
        cosM = sb.tile([L, L], F32, tag="cosM")
        cosMT = sb.tile([L, L], F32, tag="cosMT")

        att1 = _AttMax(nc, (sb, tbp, bcp, ps_bc), dins["c2rep"][:], c2x,
                       identb, cosM, jmax2, "amx2", 0)
        att2 = _AttMax(nc, (sb, tbp, bcp, ps_bc), dins["c1rep"][:], c1x,
                       identb, cosMT, jmax1, "amx1", 1)
        # ---- input loads; sync queue only carries loads + broadcasts ----
        c1T = load("c1T", [L, NCH, L], BF16, "(c p) n -> p c n", p=L)
        c2T = load("c2T", [L, NCH, L], BF16, "(c p) n -> p c n", p=L)
        rw1 = load("rw1", [L, 68])
        rw2 = load("rw2", [L, 68])
        # broadcast data for the first side-1 groups right behind them
        att1.emit_dma(2)
        load("c2x", None, t=c2x)
        load("c1x", None, t=c1x)
        invl1 = load("invl1", [L, 1])
        invl2 = load("invl2", [L, 1])
        mone1b = load("mone1b", [L, L])
        mone2b = load("mone2b", [L, L])
        c1b = load("c1b", [L, H], BF16)
        c2b = load("c2b", [L, H], BF16)
        rhs1 = load("rhs1", [L, NCH, 34], BF16, "(c p) n -> p c n", p=L)
        rhs2 = load("rhs2", [L, NCH, 34], BF16, "(c p) n -> p c n", p=L)
        w2ab = load("w2ab", [L, NCH, 34], BF16, "(c p) n -> p c n", p=L)
        rw1mp = load("rw1mp", [L, PP])
        rw2mp = load("rw2mp", [L, PP])
        mme1 = load("mme1", [L, NCH, PP], BF16, "(c p) n -> p c n", p=L)
        mme2 = load("mme2", [L, NCH, PP], BF16, "(c p) n -> p c n", p=L)

        # mm twin-matmul rhs, loaded per quarter just-in-time
        def load_mmr_qtr(name, qtr):
            t = sb.tile([L, NCH, 4 * L], BF16, tag=f"{name}q{qtr}")
            src = dins[name][:].rearrange("(c p) n -> p c n", p=L)
            qs = slice(qtr * 4 * L, (qtr + 1) * 4 * L)
            nc.sync.dma_start(t[:], src[:, :, qs])
            return t

        ident = sb.tile([L, L], F32, tag="ident")
        masks.make_identity(nc, ident[:])
        masks.make_identity(nc, identb[:])
        ones_col = sb.tile([L, 1], F32, tag="ones_col")
        nc.vector.memset(ones_col[:], 1.0)

        out1 = sb.tile([L, NF], F32, tag="out1")
        out2 = sb.tile([L, NF], F32, tag="out2")

        # ---- cos chain ----
        dotsp = ps_t.tile([L, L], F32, tag="t")
        for c in range(NCH):
            nc.tensor.matmul(dotsp[:], c1T[:, c, :], c2T[:, c, :],
                             start=(c == 0), stop=(c == NCH - 1))
        wS = sc.tile([L, L], F32, tag="wS")
        nc.vector.tensor_scalar(wS[:], dotsp[:], rw1[:, 0:1], None, OP.mult)
        wTp = ps_t.tile([L, L], F32, tag="t")
        nc.tensor.transpose(wTp[:], wS[:], ident[:])
        cosT = sb.tile([L, L], F32, tag="cosT")
        nc.vector.tensor_scalar(cosT[:], wTp[:], rw2[:, 0:1], None, OP.mult)
        cosp = ps_t.tile([L, L], F32, tag="t")
        nc.tensor.transpose(cosp[:], cosT[:], ident[:])
        cos = sb.tile([L, L], F32, tag="cos")
        nc.scalar.copy(cos[:], cosp[:])
        cosb = sb.tile([L, L], BF16, tag="cosb")
        nc.vector.tensor_copy(cosb[:], cosp[:])
        cosTb = sb.tile([L, L], BF16, tag="cosTb")
        nc.vector.tensor_copy(cosTb[:], cosT[:])

        # ---- cosM / cosMT (+1.0 in invalid columns) ----
        nc.vector.tensor_tensor(cosM[:], cosp[:], mone2b[:], op=OP.add)
        nc.vector.tensor_tensor(cosMT[:], cosT[:], mone1b[:], op=OP.add)

        # ---- attentive mean (softmax 1/sum cancels in cosine features) ----
        def att_mean(lhsT, rhs, tag):
            sp = ps_bc.tile([L, H], F32, tag="bc")
            nc.tensor.matmul(sp[:, 0:512], lhsT[:], rhs[:, 0:512],
                             start=True, stop=True)
            nc.tensor.matmul(sp[:, 512:H], lhsT[:], rhs[:, 512:H],
                             start=True, stop=True)
            am = sb.tile([L, H], BF16, tag=tag)
            nc.scalar.activation(am[:], sp[:], AF.Exp, scale=1.0)
            return am

        am2 = att_mean(cosTb, c2b, "am2")   # [i,H]
        am1 = att_mean(cosb, c1b, "am1")    # [j,H]

        # ---- cmax / cmean ----
        def cmaxmean(cosA, cosB, invl, out):
            nc.vector.reduce_max(out[:, 0:1], cosA[:], axis=AX.X)
            mp = ps_w.tile([L, 34], F32, tag="wa")
            nc.tensor.matmul(mp[:, 0:1], cosB[:], ones_col[:], start=True,
                             stop=True)
            nc.vector.tensor_scalar(out[:, 1:2], mp[:, 0:1], invl[:, 0:1],
                                    None, OP.mult)

        cmaxmean(cos, cosT, invl2, out1)
        cmaxmean(cosT, cos, invl1, out2)

        # ---- ff/bf matvec features ----
        def ff_feats(cT, rhs, rw, out):
            ffp = ps_w.tile([L, 34], F32, tag="wa")
            for c in range(NCH):
                nc.tensor.matmul(ffp[:], cT[:, c, :], rhs[:, c, :],
                                 start=(c == 0), stop=(c == NCH - 1))
            nc.vector.tensor_tensor(out[:, 2:36], ffp[:], rw[:, 0:34],
                                    op=OP.mult)

        ff_feats(c1T, rhs1, rw1, out1)
        ff_feats(c2T, rhs2, rw2, out2)

        # ---- phase-split mpm feature block ----
        def mpm_p1(v, cT, blk, tag):
            vsqT = sc.tile([L, NCH, L], BF16, tag="vsqT")
            prT = sc.tile([L, NCH, L], BF16, tag="prT")
            for c in range(NCH):
                tp = ps_t.tile([L, L], BF16, tag="t")
                nc.tensor.transpose(tp[:], v[:, c * L:(c + 1) * L], identb[:])
                nc.scalar.square(vsqT[:, c, :], tp[:])
                nc.vector.tensor_tensor(prT[:, c, :], cT[:, c, :], tp[:],
                                        op=OP.mult)
            numpt = ps_w.tile([L, 34], F32, tag="wa")
            wnpt = ps_w.tile([L, 34], F32, tag="wb")
            for c in range(NCH):
                nc.tensor.matmul(numpt[:, 0:17], prT[:, c, :],
                                 w2ab[:, c, blk],
                                 start=(c == 0), stop=(c == NCH - 1))
            for c in range(NCH):
                nc.tensor.matmul(wnpt[:, 0:17], vsqT[:, c, :],
                                 w2ab[:, c, blk],
                                 start=(c == 0), stop=(c == NCH - 1))
            st = sb.tile([L, 34], F32, tag=tag + "_st")
            nc.vector.tensor_copy(st[:, 0:17], numpt[:, 0:17])
            nc.vector.tensor_copy(st[:, 17:34], wnpt[:, 0:17])
            return st

        def mpm_p2(st, rw_side, blk, out, col0):
            rwv = sc.tile([L, 17], F32, tag="rwv")
            nc.scalar.sqrt(rwv[:], st[:, 17:34])
            nc.vector.tensor_scalar(rwv[:], rwv[:], EPS, None, OP.max)
            nc.vector.reciprocal(rwv[:], rwv[:])
            ft = sc.tile([L, 17], F32, tag="ft")
            nc.vector.tensor_tensor(ft[:], st[:, 0:17],
                                    rw_side[:, 34 + blk.start:34 + blk.stop],
                                    op=OP.mult)
            nc.vector.tensor_tensor(out[:, col0:col0 + 17], ft[:], rwv[:],
                                    op=OP.mult)

        # ---- mm (pairwise multi-perspective) twin-matmul block ----
        def mm_qtr(cT, mmrq, rwmp, out, qtr):
            o = ps_mm.tile([L, 4, L], F32, tag="mm")
            for c in range(NCH):
                nc.tensor.matmul(o[:], cT[:, c, :], mmrq[:, c, :],
                                 start=(c == 0), stop=(c == NCH - 1))
            mx = sc.tile([L, 4], F32, tag="mx4")
            nc.vector.reduce_max(mx[:, :, None], o[:], axis=AX.X)
            nc.vector.tensor_tensor(
                out[:, 36 + qtr * 4:36 + (qtr + 1) * 4], mx[:],
                rwmp[:, qtr * 4:(qtr + 1) * 4], op=OP.mult)

        def mm_mean(cT, mme, rwmp, out):
            mnt = ps_w.tile([L, 34], F32, tag="wa")
            for c in range(NCH):
                nc.tensor.matmul(mnt[:, 0:PP], cT[:, c, :], mme[:, c, :],
                                 start=(c == 0), stop=(c == NCH - 1))
            nc.vector.tensor_tensor(out[:, 52:68], mnt[:, 0:PP], rwmp[:],
                                    op=OP.mult)

        # ---- interleaved emission ----
        att1.emit(2)
        st_am2 = mpm_p1(am2, c1T, BLK_ATT, "am2")
        att1.emit(1)
        mpm_p2(st_am2, rw1, BLK_ATT, out1, 68)
        att1.emit(1)
        st_am1 = mpm_p1(am1, c2T, BLK_ATT, "am1")
        att1.emit(1)
        mr10 = load_mmr_qtr("mmr1", 0)
        mpm_p2(st_am1, rw2, BLK_ATT, out2, 68)
        att1.emit(1)
        mr11 = load_mmr_qtr("mmr1", 1)
        mm_qtr(c1T, mr10, rw1mp, out1, 0)
        att1.emit(2)
        mr12 = load_mmr_qtr("mmr1", 2)
        mm_qtr(c1T, mr11, rw1mp, out1, 1)
        att1.emit(2)
        mr13 = load_mmr_qtr("mmr1", 3)
        mm_qtr(c1T, mr12, rw1mp, out1, 2)
        att1.emit(2)
        mm_qtr(c1T, mr13, rw1mp, out1, 3)
        att1.emit(2)
        mm_mean(c1T, mme1, rw1mp, out1)
        att1.emit(99)

        att2.emit(2)
        amx2 = att1.fold()
        att2.emit(2)
        st_x2 = mpm_p1(amx2, c1T, BLK_MATT, "amx2")
        att2.emit(1)
        mr20 = load_mmr_qtr("mmr2", 0)
        mpm_p2(st_x2, rw1, BLK_MATT, out1, 85)
        att2.emit(1)
        mr21 = load_mmr_qtr("mmr2", 1)
        mm_qtr(c2T, mr20, rw2mp, out2, 0)
        att2.emit(2)
        mr22 = load_mmr_qtr("mmr2", 2)
        mm_qtr(c2T, mr21, rw2mp, out2, 1)
        att2.emit(2)
        mr23 = load_mmr_qtr("mmr2", 3)
        mm_qtr(c2T, mr22, rw2mp, out2, 2)
        att2.emit(2)
        mm_qtr(c2T, mr23, rw2mp, out2, 3)
        att2.emit(2)
        mm_mean(c2T, mme2, rw2mp, out2)
        att2.emit(99)

        # chunked fold of side 2 overlapped with its mpm consumers
        amx1 = sb.tile([L, H], BF16, tag="amx1_o")
        for c in range(NCH):
            att2.fold_chunk(amx1, c)
        st_x1 = mpm_p1(amx1, c2T, BLK_MATT, "amx1")
        mpm_p2(st_x1, rw2, BLK_MATT, out2, 85)

        # ---- store ----
        nc.sync.dma_start(dout[0:L, 0:85], out1[:, 0:85])
        nc.sync.dma_start(dout[0:L, 85:NF], out1[:, 85:NF])
        nc.sync.dma_start(dout[L:2 * L, 0:85], out2[:, 0:85])
        nc.sync.dma_start(dout[L:2 * L, 85:NF], out2[:, 85:NF])


_CACHED = {}


def _build(jmax1, jmax2):
    key = (jmax1, jmax2)
    if key in _CACHED:
        return _CACHED[key]
    nc = bacc.Bacc("TRN2", target_bir_lowering=False, debug=False,
                   enable_asserts=False)
    dins = {}
    for name, shape, dt in [
            ("c1b", [L, H], BF16), ("c2b", [L, H], BF16),
            ("c1x", [L, H], BF16), ("c2x", [L, H], BF16),
            ("c1T", [H, L], BF16), ("c2T", [H, L], BF16),
            ("c1rep", [L, L * H], BF16), ("c2rep", [L, L * H], BF16),
            ("rhs1", [H, 34], BF16), ("rhs2", [H, 34], BF16),
            ("w2ab", [H, 34], BF16),
            ("mmr1", [H, PP * L], BF16), ("mmr2", [H, PP * L], BF16),
            ("mme1", [H, PP], BF16), ("mme2", [H, PP], BF16),
            ("rw1", [L, 68], F32), ("rw2", [L, 68], F32),
            ("rw1mp", [L, PP], F32), ("rw2mp", [L, PP], F32),
            ("mone1b", [L, L], F32), ("mone2b", [L, L], F32),
            ("invl1", [L, 1], F32), ("invl2", [L, 1], F32)]:
        dins[name] = nc.dram_tensor(name, shape, dt, kind="ExternalInput")
    dout = nc.dram_tensor("out", [2 * L, NF], F32, kind="ExternalOutput")
    with tile.TileContext(nc) as tc:
        _trace_kernel(tc, dins, dout[:], jmax1, jmax2)
    nc.compile()
    _CACHED[key] = nc
    return nc


def _host_prep(c1raw, m1, c2raw, m2, w_ff, w_fb, w_mp, w_att, w_matt):
    asb = lambda a: np.ascontiguousarray(a, dtype=ml_dtypes.bfloat16)
    asf = lambda a: np.ascontiguousarray(a, dtype=np.float32)

    c1 = (c1raw * m1[:, None]).astype(np.float32)
    c2 = (c2raw * m2[:, None]).astype(np.float32)
    len1, len2 = float(m1.sum()), float(m2.sum())
    lp1, lp2 = max(int(len1) - 1, 0), max(int(len2) - 1, 0)

    def mpm_rhs(v, w):
        w2 = w * w
        rn = 1.0 / max(np.sqrt((v * v).sum()), EPS)
        wn = np.sqrt((w2 * (v * v)[None, :]).sum(1))
        rwn = 1.0 / np.maximum(wn, EPS)
        return np.concatenate(
            [(v * rn)[:, None], (w2 * v[None, :] * rwn[:, None]).T], 1)

    rhs1 = np.concatenate([mpm_rhs(c2[lp2], w_ff), mpm_rhs(c2[0], w_fb)], 1)
    rhs2 = np.concatenate([mpm_rhs(c1[lp1], w_ff), mpm_rhs(c1[0], w_fb)], 1)

    # rw tables [L, 68]: [ones|ff16 | ones|fb16 | ones|att16 | ones|matt16]
    ones_h = np.ones((1, H), np.float32)
    wsq = np.concatenate([ones_h, w_ff**2, ones_h, w_fb**2,
                          ones_h, w_att**2, ones_h, w_matt**2], 0)  # [68,H]
    def rw_of(c):
        wn = np.sqrt(wsq @ (c * c).T)           # [68, L]
        return (1.0 / np.maximum(wn, EPS)).T    # [L, 68]
    rw1, rw2 = rw_of(c1), rw_of(c2)

    # mp folding
    w2mp = (w_mp * w_mp).astype(np.float32)          # [P, H]
    def rwmp_of(c):
        wn = np.sqrt(w2mp @ (c * c).T)               # [P, L]
        return 1.0 / np.maximum(wn, EPS)             # [P, L]
    rw1mp_t, rw2mp_t = rwmp_of(c1), rwmp_of(c2)      # [P, L]
    # mmr1[h, (p, j)] = w2mp[p,h] * c2[j,h] * rw2mp[p,j]
    mmr1 = np.einsum("ph,jh,pj->hpj", w2mp, c2, rw2mp_t).reshape(H, PP * L)
    mmr2 = np.einsum("ph,ih,pi->hpi", w2mp, c1, rw1mp_t).reshape(H, PP * L)
    # mme1[h, p] = w2mp[p,h] * (sum_j c2[j,h] rw2mp[p,j]) / len2
    s2 = np.einsum("jh,pj->ph", c2, rw2mp_t)
    s1 = np.einsum("ih,pi->ph", c1, rw1mp_t)
    mme1 = (w2mp * s2 / max(len2, EPS)).T            # [H, P]
    mme2 = (w2mp * s1 / max(len1, EPS)).T

    w2ab = np.concatenate([ones_h, w_att**2, ones_h, w_matt**2], 0).T  # [H,34]

    # broadcast-replicated masked rows: every partition holds every row
    c1x = asb(c1 + ((m1 - 1) * 1e30)[:, None])
    c2x = asb(c2 + ((m2 - 1) * 1e30)[:, None])
    c1rep = np.ascontiguousarray(
        np.broadcast_to(c1x.reshape(-1)[None, :], (L, L * H)))
    c2rep = np.ascontiguousarray(
        np.broadcast_to(c2x.reshape(-1)[None, :], (L, L * H)))

    bc = lambda r: np.ascontiguousarray(
        np.broadcast_to(r[None, :], (L, L)), dtype=np.float32)
    return dict(
        c1b=asb(c1), c2b=asb(c2), c1x=c1x, c2x=c2x,
        c1rep=c1rep, c2rep=c2rep,
        c1T=asb(c1.T), c2T=asb(c2.T),
        rhs1=asb(rhs1), rhs2=asb(rhs2), w2ab=asb(w2ab),
        mmr1=asb(mmr1), mmr2=asb(mmr2), mme1=asb(mme1), mme2=asb(mme2),
        rw1=asf(rw1), rw2=asf(rw2),
        rw1mp=asf(rw1mp_t.T), rw2mp=asf(rw2mp_t.T),
        mone1b=bc(1 - m1), mone2b=bc(1 - m2),
        invl1=np.full((L, 1), 1.0 / max(len1, EPS), np.float32),
        invl2=np.full((L, 1), 1.0 / max(len2, EPS), np.float32),
    )


def kernel(context_1, mask_1, context_2, mask_2,
           w_ff, w_fb, w_mp, w_att, w_matt, **_unused):
    context_1 = np.asarray(context_1, dtype=np.float32)
    context_2 = np.asarray(context_2, dtype=np.float32)
    mask_1 = np.asarray(mask_1, dtype=np.float32)
    mask_2 = np.asarray(mask_2, dtype=np.float32)
    w_ff, w_fb = np.asarray(w_ff, np.float32), np.asarray(w_fb, np.float32)
    w_mp = np.asarray(w_mp, np.float32)
    w_att, w_matt = np.asarray(w_att, np.float32), np.asarray(w_matt, np.float32)
    assert context_1.shape == (B, L, H), context_1.shape

    jmax1 = int(mask_1.sum(1).max())
    jmax2 = int(mask_2.sum(1).max())
    nc = _build(jmax1, jmax2)
    in_maps = [
        _host_prep(context_1[b], mask_1[b], context_2[b], mask_2[b],
                   w_ff, w_fb, w_mp, w_att, w_matt)
        for b in range(B)
    ]
    res = run_bass_kernel_spmd(nc, in_maps, core_ids=list(range(B)))
    global LAST_RESULTS
    LAST_RESULTS = res
    return np.stack([res.results[b]["out"] for b in range(B)]).astype(np.float32)


LAST_RESULTS = None
